# revision 19
# baseline (speedup 1.0000x reference)
"""Trainium2 Bass kernel for the soft neural decision tree (moe_routing).

Math (per batch row b):
  z~[q]   = sign(q) * (Wr[node(q)] . x[b])            q = (level, leaf) expanded
  path[l] = prod_level sigma(z~)                      (1 - sigma(z) = sigma(-z))
  h       = relu(x @ W1cat)                           W1cat in (j, l) interleaved order
  hw      = h * path[l(k')]                           broadcast over j
  out     = hw @ W2cat + path @ b2

Sharding: data-parallel over batch, 8 cores x 4096 rows. All GEMM inputs are
fp16 (1 cycle/row on the PE; fp32 is 4). The router/path stage for chunk
nb+1 is emitted mid-chunk so its latency chain hides under GEMM work.
PSUM evacuation of h is split between ScalarE (fused relu+cast) and VectorE
(fused scalar_tensor_tensor (h max 0) * path) to balance engines; the path
multiply runs on VectorE as fp16 tensor_tensor at 2x with a 0-stride
broadcast AP for the path operand. GEMM2 is k-split over 4 PE column-tile
groups whose partial sums are combined on the host.
"""

import numpy as np

N_CORES = 8
B = 32768
BC = B // N_CORES          # batch rows per core
NB = 8                     # batch sub-chunks per core
NBW = BC // NB             # 1024 columns per sub-chunk
D = 128
H = 64
L = 64
O = 10
DEPTH = 6
NCH = 32                   # k' chunks of 128 rows
N_STT = 4                  # cg units (of 16 per nb) evacuated via DVE STT route

_cache = {}


def _build_module(br_zero, b1_zero, b2_zero):
    from concourse import bacc, tile
    import concourse.mybir as mybir

    f16 = mybir.dt.float16
    f32 = mybir.dt.float32
    Alu = mybir.AluOpType
    Act = mybir.ActivationFunctionType

    nc = bacc.Bacc(None)
    xT_d = nc.declare_dram_parameter("xT", [D, BC], f16, isOutput=False)
    wrexp_d = nc.declare_dram_parameter("WrExp", [D, 384], f16, isOutput=False)
    w1p_d = nc.declare_dram_parameter("W1p", [D, 4096], f16, isOutput=False)
    w2p_d = nc.declare_dram_parameter("W2p", [D, NCH * O], f16, isOutput=False)
    if not b2_zero:
        b2s_d = nc.declare_dram_parameter("b2s", [H, O], f16, isOutput=False)
    if not br_zero:
        brexp_d = nc.declare_dram_parameter("brExp", [D, 3], f32, isOutput=False)
    if not b1_zero:
        b1p_d = nc.declare_dram_parameter("b1p", [D, NCH], f32, isOutput=False)
    # 4 col-tiling partial sums per output element; summed on the host
    outT_d = nc.declare_dram_parameter("outT4", [4 * O, BC], f16, isOutput=True)

    with tile.TileContext(nc) as tc:
        with (
            tc.tile_pool(name="const", bufs=1) as cpool,
            tc.tile_pool(name="s", bufs=2) as spool,
            tc.tile_pool(name="path", bufs=2) as ppool,
            tc.tile_pool(name="hr", bufs=3) as hrpool,
            tc.tile_pool(name="hw", bufs=8) as hwpool,
            tc.tile_pool(name="ot", bufs=2) as otpool,
            tc.tile_pool(name="psz", bufs=1, space="PSUM") as pszpool,
            tc.tile_pool(name="ph", bufs=2, space="PSUM") as phpool,
            tc.tile_pool(name="po", bufs=1, space="PSUM") as popool,
        ):
            xT = cpool.tile([D, BC], f16)
            wrexp = cpool.tile([D, 384], f16)
            w1p = cpool.tile([D, 4096], f16)
            w2p = cpool.tile([D, NCH * O], f16)
            nc.sync.dma_start(wrexp[:], wrexp_d[:])
            nc.sync.dma_start(xT[:, 0:NBW], xT_d[:, 0:NBW])
            for q in range(4):
                nc.sync.dma_start(w1p[:, q * 1024:(q + 1) * 1024],
                                  w1p_d[:, q * 1024:(q + 1) * 1024])
            nc.sync.dma_start(w2p[:], w2p_d[:])
            for q in range(1, NB):
                nc.sync.dma_start(xT[:, q * NBW:(q + 1) * NBW],
                                  xT_d[:, q * NBW:(q + 1) * NBW])
            if not b2_zero:
                b2s = cpool.tile([H, O], f16)
                nc.sync.dma_start(b2s[:], b2s_d[:])
            if not br_zero:
                brexp = cpool.tile([D, 3], f32)
                nc.sync.dma_start(brexp[:], brexp_d[:])
            if not b1_zero:
                b1p = cpool.tile([D, NCH], f32)
                nc.sync.dma_start(b1p[:], b1p_d[:])

            def emit_router(nb):
                """G0 + sigmoid + path products for batch chunk nb.
                Returns (w, path2): path^T [64, NBW] and its 2x partition
                replica [128, NBW], both fp16."""
                bs = slice(nb * NBW, (nb + 1) * NBW)
                psz = pszpool.tile([D, 3 * NBW], f32)
                for k in range(3):
                    nc.tensor.matmul(
                        psz[:, k * NBW:(k + 1) * NBW],
                        wrexp[:, k * D:(k + 1) * D],
                        xT[:, bs],
                        start=True, stop=True,
                    )
                s = spool.tile([D, 3 * NBW], f16)
                if br_zero:
                    nc.scalar.activation(s[:], psz[:], Act.Sigmoid)
                else:
                    for k in range(3):
                        nc.scalar.activation(
                            s[:, k * NBW:(k + 1) * NBW],
                            psz[:, k * NBW:(k + 1) * NBW],
                            Act.Sigmoid, bias=brexp[:, k:k + 1],
                        )
                # s chunk layout (host WrExp col order):
                # chunk0 = [lvl0 | lvl3], chunk1 = [lvl1 | lvl4], chunk2 = [lvl2 | lvl5]
                # DVE operands must share a start partition, so the cross-half
                # combine goes through SBUF->SBUF DMA.
                c0 = s[:, 0:NBW]
                c1 = s[:, NBW:2 * NBW]
                c2 = s[:, 2 * NBW:3 * NBW]
                t1 = ppool.tile([D, NBW], f16, tag="t1")
                nc.vector.tensor_tensor(t1[:], c0, c1, Alu.mult)     # [l01 | l34]
                t2 = ppool.tile([D, NBW], f16, tag="t2")
                nc.vector.tensor_tensor(t2[:], t1[:], c2, Alu.mult)  # [l012 | l345]
                u = ppool.tile([H, NBW], f16, tag="u")
                nc.sync.dma_start(u[:], t2[H:2 * H, :])
                w = ppool.tile([H, NBW], f16, tag="w")
                nc.vector.tensor_tensor(w[:], t2[0:H, :], u[:], Alu.mult)  # path^T
                path2 = ppool.tile([D, NBW], f16, tag="path2")
                nc.sync.dma_start(path2[0:H, :], w[:])
                nc.sync.dma_start(path2[H:2 * H, :], w[:])
                return w, path2

            cur = emit_router(0)
            for nb in range(NB):
                bs = slice(nb * NBW, (nb + 1) * NBW)
                w, path2 = cur

                # G2 is interleaved into the cg loop, delayed by G2_LAG units,
                # so PE never queues behind the full evacuation.
                G2_LAG = 2
                po = popool.tile([D, NBW], f32)

                def emit_g2_unit(cgu):
                    for c in (2 * cgu, 2 * cgu + 1):
                        g = c % 4
                        last = c >= NCH - 4 and (b2_zero or g != 0)
                        nc.tensor.matmul(
                            po[32 * g:32 * g + O, :],
                            w2p[:, O * c:O * (c + 1)],
                            hw_tiles[c // 2][:, (c & 1) * NBW:((c & 1) + 1) * NBW],
                            start=(c < 4),
                            stop=last,
                            tile_position=(0, 32 * g),
                        )

                # G1 (+ fused evacuation) over 16 cg units = 32 k' chunks
                hw_tiles = []
                for cg in range(16):
                    ph = phpool.tile([D, 2 * NBW], f32)
                    for half in range(2):
                        c = 2 * cg + half
                        nc.tensor.matmul(
                            ph[:, half * NBW:(half + 1) * NBW],
                            w1p[:, c * D:(c + 1) * D],
                            xT[:, bs],
                            start=True, stop=True,
                        )
                    if cg == 3 and nb + 1 < NB:
                        nxt = emit_router(nb + 1)
                    hw = hwpool.tile([D, 2 * NBW], f16)
                    hw_tiles.append(hw)
                    # [D, 2, NBW] views so path2 broadcasts along the pair dim
                    path2b = path2[:].rearrange(
                        "p (one f) -> p one f", one=1).broadcast_to([D, 2, NBW])
                    hw3 = hw[:].rearrange("p (a f) -> p a f", a=2)
                    ph3 = ph[:].rearrange("p (a f) -> p a f", a=2)
                    if b1_zero and cg < N_STT:
                        # DVE route: hw = (h max 0) * path2, straight from PSUM
                        nc.vector.scalar_tensor_tensor(
                            hw3, ph3, 0.0, path2b, Alu.max, Alu.mult,
                        )
                    else:
                        # ACT route: relu(+b1) cast to fp16, then DVE multiply
                        hr = hrpool.tile([D, 2 * NBW], f16)
                        if b1_zero:
                            nc.scalar.activation(hr[:], ph[:], Act.Relu)
                        else:
                            for half in range(2):
                                c = 2 * cg + half
                                nc.scalar.activation(
                                    hr[:, half * NBW:(half + 1) * NBW],
                                    ph[:, half * NBW:(half + 1) * NBW],
                                    Act.Relu, bias=b1p[:, c:c + 1],
                                )
                        hr3 = hr[:].rearrange("p (a f) -> p a f", a=2)
                        nc.vector.tensor_tensor(hw3, hr3, path2b, Alu.mult)
                    if cg >= G2_LAG:
                        emit_g2_unit(cg - G2_LAG)
                for cgu in range(16 - G2_LAG, 16):
                    emit_g2_unit(cgu)
                if not b2_zero:
                    nc.tensor.matmul(
                        po[0:O, :], b2s[:], w[:],
                        start=False, stop=True, tile_position=(0, 0),
                    )

                # partition-aligned PSUM->SBUF copies (fp16), 4 DMAs out
                ot = otpool.tile([D, NBW], f16)
                for g in range(4):
                    nc.vector.tensor_copy(
                        ot[32 * g:32 * g + O, :],
                        po[32 * g:32 * g + O, :],
                    )
                for g in range(4):
                    nc.sync.dma_start(
                        outT_d[O * g:O * (g + 1), bs],
                        ot[32 * g:32 * g + O, :],
                    )
                if nb + 1 < NB:
                    cur = nxt

    nc.compile()
    return nc


def _prep(x, Wr, br, W1, b1, W2, b2):
    """Host-side operand preparation (all fp16 except biases)."""
    chunk_levels = [(0, 3), (1, 4), (2, 5)]
    WrExp = np.zeros((D, 384), np.float32)
    brExp = np.zeros(384, np.float32)
    for k, levels in enumerate(chunk_levels):
        for half, lev in enumerate(levels):
            for l in range(L):
                q = 128 * k + 64 * half + l
                node = (2 ** lev - 1) + (l >> (DEPTH - lev))
                bit = (l >> (DEPTH - 1 - lev)) & 1
                sign = 1.0 - 2.0 * bit
                WrExp[:, q] = sign * Wr[node, :]
                brExp[q] = sign * br[node]

    W1p = W1.transpose(1, 2, 0).reshape(D, 4096)       # [d, j*64+l]
    W2p = W2.transpose(1, 0, 2).reshape(4096, O)       # [j*64+l, o]
    W2sb = W2p.reshape(NCH, D, O).transpose(1, 0, 2).reshape(D, NCH * O)
    b1p = b1.T.reshape(4096)                           # [j*64+l]

    shared = {
        "WrExp": np.ascontiguousarray(WrExp.astype(np.float16)),
        "W1p": np.ascontiguousarray(W1p.astype(np.float16)),
        "W2p": np.ascontiguousarray(W2sb.astype(np.float16)),
    }
    flags = (not np.any(brExp), not np.any(b1p), not np.any(b2))
    br_zero, b1_zero, b2_zero = flags
    if not b2_zero:
        shared["b2s"] = np.ascontiguousarray(b2.astype(np.float16))
    if not br_zero:
        shared["brExp"] = np.ascontiguousarray(brExp.reshape(3, D).T.astype(np.float32))
    if not b1_zero:
        shared["b1p"] = np.ascontiguousarray(b1p.reshape(NCH, D).T.astype(np.float32))
    return shared, flags


def kernel(x, Wr, br, W1, b1, W2, b2):
    from concourse.bass_utils import run_bass_kernel_spmd

    x = np.asarray(x, np.float32)
    Wr = np.asarray(Wr, np.float32)
    br = np.asarray(br, np.float32)
    W1 = np.asarray(W1, np.float32)
    b1 = np.asarray(b1, np.float32)
    W2 = np.asarray(W2, np.float32)
    b2 = np.asarray(b2, np.float32)

    shared, flags = _prep(x, Wr, br, W1, b1, W2, b2)
    if flags not in _cache:
        _cache[flags] = _build_module(*flags)
    nc = _cache[flags]

    in_maps = []
    for i in range(N_CORES):
        m = dict(shared)
        m["xT"] = np.ascontiguousarray(
            x[i * BC:(i + 1) * BC].T.astype(np.float16)
        )
        in_maps.append(m)

    res = run_bass_kernel_spmd(nc, in_maps, list(range(N_CORES)))
    out = np.empty((B, O), np.float32)
    for i in range(N_CORES):
        p4 = res.results[i]["outT4"].astype(np.float32)
        out[i * BC:(i + 1) * BC] = p4.reshape(4, O, BC).sum(axis=0).T
    return out


# revision 23
# speedup vs baseline: 1.0018x; 1.0018x over previous
"""Trainium2 Bass kernel for the soft neural decision tree (moe_routing).

Math (per batch row b):
  z~[q]   = sign(q) * (Wr[node(q)] . x[b])            q = (level, leaf) expanded
  path[l] = prod_level sigma(z~)                      (1 - sigma(z) = sigma(-z))
  h       = relu(x @ W1cat)                           W1cat in (j, l) interleaved order
  hw      = h * path[l(k')]                           broadcast over j
  out     = hw @ W2cat + path @ b2

Sharding: data-parallel over batch, 8 cores x 4096 rows. All GEMM inputs are
fp16 (1 cycle/row on the PE; fp32 is 4). The router/path stage for chunk
nb+1 is emitted mid-chunk so its latency chain hides under GEMM work.
PSUM evacuation of h is split between ScalarE (fused relu+cast) and VectorE
(fused scalar_tensor_tensor (h max 0) * path) to balance engines; the path
multiply runs on VectorE as fp16 tensor_tensor at 2x with a 0-stride
broadcast AP for the path operand. GEMM2 is k-split over 4 PE column-tile
groups whose partial sums are combined on the host.
"""

import numpy as np

N_CORES = 8
B = 32768
BC = B // N_CORES          # batch rows per core
NB = 8                     # batch sub-chunks per core
NBW = BC // NB             # 1024 columns per sub-chunk
D = 128
H = 64
L = 64
O = 10
DEPTH = 6
NCH = 32                   # k' chunks of 128 rows
N_STT = 5                  # cg units (of 16 per nb) evacuated via DVE STT route
# spread the DVE-routed units through the loop so ACT and DVE overlap
STT_SET = {(i * 16) // N_STT + 1 for i in range(N_STT)}

_cache = {}


def _build_module(br_zero, b1_zero, b2_zero):
    from concourse import bacc, tile
    import concourse.mybir as mybir

    f16 = mybir.dt.float16
    f32 = mybir.dt.float32
    Alu = mybir.AluOpType
    Act = mybir.ActivationFunctionType

    nc = bacc.Bacc(None)
    xT_d = nc.declare_dram_parameter("xT", [D, BC], f16, isOutput=False)
    wrexp_d = nc.declare_dram_parameter("WrExp", [D, 384], f16, isOutput=False)
    w1p_d = nc.declare_dram_parameter("W1p", [D, 4096], f16, isOutput=False)
    w2p_d = nc.declare_dram_parameter("W2p", [D, NCH * O], f16, isOutput=False)
    if not b2_zero:
        b2s_d = nc.declare_dram_parameter("b2s", [H, O], f16, isOutput=False)
    if not br_zero:
        brexp_d = nc.declare_dram_parameter("brExp", [D, 3], f32, isOutput=False)
    if not b1_zero:
        b1p_d = nc.declare_dram_parameter("b1p", [D, NCH], f32, isOutput=False)
    # 4 col-tiling partial sums per output element; summed on the host
    outT_d = nc.declare_dram_parameter("outT4", [4 * O, BC], f16, isOutput=True)

    with tile.TileContext(nc) as tc:
        with (
            tc.tile_pool(name="const", bufs=1) as cpool,
            tc.tile_pool(name="s", bufs=2) as spool,
            tc.tile_pool(name="path", bufs=2) as ppool,
            tc.tile_pool(name="hr", bufs=3) as hrpool,
            tc.tile_pool(name="hw", bufs=8) as hwpool,
            tc.tile_pool(name="ot", bufs=2) as otpool,
            tc.tile_pool(name="psz", bufs=1, space="PSUM") as pszpool,
            tc.tile_pool(name="ph", bufs=2, space="PSUM") as phpool,
            tc.tile_pool(name="po", bufs=1, space="PSUM") as popool,
        ):
            xT = cpool.tile([D, BC], f16)
            wrexp = cpool.tile([D, 384], f16)
            w1p = cpool.tile([D, 4096], f16)
            w2p = cpool.tile([D, NCH * O], f16)
            nc.sync.dma_start(wrexp[:], wrexp_d[:])
            nc.sync.dma_start(xT[:, 0:NBW], xT_d[:, 0:NBW])
            for q in range(4):
                nc.sync.dma_start(w1p[:, q * 1024:(q + 1) * 1024],
                                  w1p_d[:, q * 1024:(q + 1) * 1024])
            nc.sync.dma_start(w2p[:], w2p_d[:])
            for q in range(1, NB):
                nc.sync.dma_start(xT[:, q * NBW:(q + 1) * NBW],
                                  xT_d[:, q * NBW:(q + 1) * NBW])
            if not b2_zero:
                b2s = cpool.tile([H, O], f16)
                nc.sync.dma_start(b2s[:], b2s_d[:])
            if not br_zero:
                brexp = cpool.tile([D, 3], f32)
                nc.sync.dma_start(brexp[:], brexp_d[:])
            if not b1_zero:
                b1p = cpool.tile([D, NCH], f32)
                nc.sync.dma_start(b1p[:], b1p_d[:])

            def emit_router(nb):
                """G0 + sigmoid + path products for batch chunk nb.
                Returns (w, path2): path^T [64, NBW] and its 2x partition
                replica [128, NBW], both fp16."""
                bs = slice(nb * NBW, (nb + 1) * NBW)
                psz = pszpool.tile([D, 3 * NBW], f32)
                for k in range(3):
                    nc.tensor.matmul(
                        psz[:, k * NBW:(k + 1) * NBW],
                        wrexp[:, k * D:(k + 1) * D],
                        xT[:, bs],
                        start=True, stop=True,
                    )
                s = spool.tile([D, 3 * NBW], f16)
                if br_zero:
                    nc.scalar.activation(s[:], psz[:], Act.Sigmoid)
                else:
                    for k in range(3):
                        nc.scalar.activation(
                            s[:, k * NBW:(k + 1) * NBW],
                            psz[:, k * NBW:(k + 1) * NBW],
                            Act.Sigmoid, bias=brexp[:, k:k + 1],
                        )
                # s chunk layout (host WrExp col order):
                # chunk0 = [lvl0 | lvl3], chunk1 = [lvl1 | lvl4], chunk2 = [lvl2 | lvl5]
                # DVE operands must share a start partition, so the cross-half
                # combine goes through SBUF->SBUF DMA.
                c0 = s[:, 0:NBW]
                c1 = s[:, NBW:2 * NBW]
                c2 = s[:, 2 * NBW:3 * NBW]
                t1 = ppool.tile([D, NBW], f16, tag="t1")
                nc.gpsimd.tensor_tensor(t1[:], c0, c1, Alu.mult)     # [l01 | l34]
                t2 = ppool.tile([D, NBW], f16, tag="t2")
                nc.gpsimd.tensor_tensor(t2[:], t1[:], c2, Alu.mult)  # [l012 | l345]
                u = ppool.tile([H, NBW], f16, tag="u")
                nc.sync.dma_start(u[:], t2[H:2 * H, :])
                w = ppool.tile([H, NBW], f16, tag="w")
                nc.gpsimd.tensor_tensor(w[:], t2[0:H, :], u[:], Alu.mult)  # path^T
                path2 = ppool.tile([D, NBW], f16, tag="path2")
                nc.sync.dma_start(path2[0:H, :], w[:])
                nc.sync.dma_start(path2[H:2 * H, :], w[:])
                return w, path2

            cur = emit_router(0)
            for nb in range(NB):
                bs = slice(nb * NBW, (nb + 1) * NBW)
                w, path2 = cur

                # G2 is interleaved into the cg loop, delayed by G2_LAG units,
                # so PE never queues behind the full evacuation.
                G2_LAG = 2
                po = popool.tile([D, NBW], f32)

                def emit_g2_unit(cgu):
                    for c in (2 * cgu, 2 * cgu + 1):
                        g = c % 4
                        last = c >= NCH - 4 and (b2_zero or g != 0)
                        nc.tensor.matmul(
                            po[32 * g:32 * g + O, :],
                            w2p[:, O * c:O * (c + 1)],
                            hw_tiles[c // 2][:, (c & 1) * NBW:((c & 1) + 1) * NBW],
                            start=(c < 4),
                            stop=last,
                            tile_position=(0, 32 * g),
                        )

                # G1 (+ fused evacuation) over 16 cg units = 32 k' chunks
                hw_tiles = []
                for cg in range(16):
                    ph = phpool.tile([D, 2 * NBW], f32)
                    for half in range(2):
                        c = 2 * cg + half
                        nc.tensor.matmul(
                            ph[:, half * NBW:(half + 1) * NBW],
                            w1p[:, c * D:(c + 1) * D],
                            xT[:, bs],
                            start=True, stop=True,
                        )
                    if cg == 3 and nb + 1 < NB:
                        nxt = emit_router(nb + 1)
                    hw = hwpool.tile([D, 2 * NBW], f16)
                    hw_tiles.append(hw)
                    # [D, 2, NBW] views so path2 broadcasts along the pair dim
                    path2b = path2[:].rearrange(
                        "p (one f) -> p one f", one=1).broadcast_to([D, 2, NBW])
                    hw3 = hw[:].rearrange("p (a f) -> p a f", a=2)
                    ph3 = ph[:].rearrange("p (a f) -> p a f", a=2)
                    if b1_zero and cg in STT_SET:
                        # DVE route: hw = (h max 0) * path2, straight from PSUM
                        nc.vector.scalar_tensor_tensor(
                            hw3, ph3, 0.0, path2b, Alu.max, Alu.mult,
                        )
                    else:
                        # ACT route: relu(+b1) cast to fp16, then DVE multiply
                        hr = hrpool.tile([D, 2 * NBW], f16)
                        if b1_zero:
                            nc.scalar.activation(hr[:], ph[:], Act.Relu)
                        else:
                            for half in range(2):
                                c = 2 * cg + half
                                nc.scalar.activation(
                                    hr[:, half * NBW:(half + 1) * NBW],
                                    ph[:, half * NBW:(half + 1) * NBW],
                                    Act.Relu, bias=b1p[:, c:c + 1],
                                )
                        hr3 = hr[:].rearrange("p (a f) -> p a f", a=2)
                        nc.vector.tensor_tensor(hw3, hr3, path2b, Alu.mult)
                    if cg >= G2_LAG:
                        emit_g2_unit(cg - G2_LAG)
                for cgu in range(16 - G2_LAG, 16):
                    emit_g2_unit(cgu)
                if not b2_zero:
                    nc.tensor.matmul(
                        po[0:O, :], b2s[:], w[:],
                        start=False, stop=True, tile_position=(0, 0),
                    )

                # partition-aligned PSUM->SBUF copies (fp16), 4 DMAs out
                ot = otpool.tile([D, NBW], f16)
                for g in range(4):
                    nc.scalar.activation(
                        ot[32 * g:32 * g + O, :],
                        po[32 * g:32 * g + O, :],
                        Act.Copy,
                    )
                for g in range(4):
                    nc.sync.dma_start(
                        outT_d[O * g:O * (g + 1), bs],
                        ot[32 * g:32 * g + O, :],
                    )
                if nb + 1 < NB:
                    cur = nxt

    nc.compile()
    return nc


def _prep(x, Wr, br, W1, b1, W2, b2):
    """Host-side operand preparation (all fp16 except biases)."""
    chunk_levels = [(0, 3), (1, 4), (2, 5)]
    WrExp = np.zeros((D, 384), np.float32)
    brExp = np.zeros(384, np.float32)
    for k, levels in enumerate(chunk_levels):
        for half, lev in enumerate(levels):
            for l in range(L):
                q = 128 * k + 64 * half + l
                node = (2 ** lev - 1) + (l >> (DEPTH - lev))
                bit = (l >> (DEPTH - 1 - lev)) & 1
                sign = 1.0 - 2.0 * bit
                WrExp[:, q] = sign * Wr[node, :]
                brExp[q] = sign * br[node]

    W1p = W1.transpose(1, 2, 0).reshape(D, 4096)       # [d, j*64+l]
    W2p = W2.transpose(1, 0, 2).reshape(4096, O)       # [j*64+l, o]
    W2sb = W2p.reshape(NCH, D, O).transpose(1, 0, 2).reshape(D, NCH * O)
    b1p = b1.T.reshape(4096)                           # [j*64+l]

    shared = {
        "WrExp": np.ascontiguousarray(WrExp.astype(np.float16)),
        "W1p": np.ascontiguousarray(W1p.astype(np.float16)),
        "W2p": np.ascontiguousarray(W2sb.astype(np.float16)),
    }
    flags = (not np.any(brExp), not np.any(b1p), not np.any(b2))
    br_zero, b1_zero, b2_zero = flags
    if not b2_zero:
        shared["b2s"] = np.ascontiguousarray(b2.astype(np.float16))
    if not br_zero:
        shared["brExp"] = np.ascontiguousarray(brExp.reshape(3, D).T.astype(np.float32))
    if not b1_zero:
        shared["b1p"] = np.ascontiguousarray(b1p.reshape(NCH, D).T.astype(np.float32))
    return shared, flags


def kernel(x, Wr, br, W1, b1, W2, b2):
    from concourse.bass_utils import run_bass_kernel_spmd

    x = np.asarray(x, np.float32)
    Wr = np.asarray(Wr, np.float32)
    br = np.asarray(br, np.float32)
    W1 = np.asarray(W1, np.float32)
    b1 = np.asarray(b1, np.float32)
    W2 = np.asarray(W2, np.float32)
    b2 = np.asarray(b2, np.float32)

    shared, flags = _prep(x, Wr, br, W1, b1, W2, b2)
    if flags not in _cache:
        _cache[flags] = _build_module(*flags)
    nc = _cache[flags]

    in_maps = []
    for i in range(N_CORES):
        m = dict(shared)
        m["xT"] = np.ascontiguousarray(
            x[i * BC:(i + 1) * BC].T.astype(np.float16)
        )
        in_maps.append(m)

    res = run_bass_kernel_spmd(nc, in_maps, list(range(N_CORES)))
    out = np.empty((B, O), np.float32)
    for i in range(N_CORES):
        p4 = res.results[i]["outT4"].astype(np.float32)
        out[i * BC:(i + 1) * BC] = p4.reshape(4, O, BC).sum(axis=0).T
    return out


# revision 24
# speedup vs baseline: 1.0421x; 1.0403x over previous
"""Trainium2 Bass kernel for the soft neural decision tree (moe_routing).

Math (per batch row b):
  z~[q]   = sign(q) * (Wr[node(q)] . x[b])            q = (level, leaf) expanded
  path[l] = prod_level sigma(z~)                      (1 - sigma(z) = sigma(-z))
  h       = relu(x @ W1cat)                           W1cat in (j, l) interleaved order
  hw      = h * path[l(k')]                           broadcast over j
  out     = hw @ W2cat + path @ b2

Sharding: data-parallel over batch, 8 cores x 4096 rows. All GEMM inputs are
fp16 (1 cycle/row on the PE; fp32 is 4). The router/path stage for chunk
nb+1 is emitted mid-chunk so its latency chain hides under GEMM work.
PSUM evacuation of h is split between ScalarE (fused relu+cast) and VectorE
(fused scalar_tensor_tensor (h max 0) * path) to balance engines; the path
multiply runs on VectorE as fp16 tensor_tensor at 2x with a 0-stride
broadcast AP for the path operand. GEMM2 is k-split over 4 PE column-tile
groups whose partial sums are combined on the host.
"""

import numpy as np

N_CORES = 8
B = 32768
BC = B // N_CORES          # batch rows per core
NB = 8                     # batch sub-chunks per core
NBW = BC // NB             # 1024 columns per sub-chunk
D = 128
H = 64
L = 64
O = 10
DEPTH = 6
NCH = 32                   # k' chunks of 128 rows
N_STT = 5                  # cg units (of 16 per nb) evacuated via DVE STT route
# spread the DVE-routed units through the loop so ACT and DVE overlap
STT_SET = {(i * 16) // N_STT + 1 for i in range(N_STT)}

_cache = {}


def _build_module(br_zero, b1_zero, b2_zero):
    from concourse import bacc, tile
    import concourse.mybir as mybir

    f16 = mybir.dt.float16
    f32 = mybir.dt.float32
    Alu = mybir.AluOpType
    Act = mybir.ActivationFunctionType

    nc = bacc.Bacc(None)
    xT_d = nc.declare_dram_parameter("xT", [D, BC], f16, isOutput=False)
    wrexp_d = nc.declare_dram_parameter("WrExp", [D, 384], f16, isOutput=False)
    w1p_d = nc.declare_dram_parameter("W1p", [D, 4096], f16, isOutput=False)
    w2p_d = nc.declare_dram_parameter("W2p", [D, NCH * O], f16, isOutput=False)
    if not b2_zero:
        b2s_d = nc.declare_dram_parameter("b2s", [H, O], f16, isOutput=False)
    if not br_zero:
        brexp_d = nc.declare_dram_parameter("brExp", [D, 3], f32, isOutput=False)
    if not b1_zero:
        b1p_d = nc.declare_dram_parameter("b1p", [D, NCH], f32, isOutput=False)
    # 4 col-tiling partial sums per output element; summed on the host
    outT_d = nc.declare_dram_parameter("outT4", [4 * O, BC], f16, isOutput=True)

    with tile.TileContext(nc) as tc:
        with (
            tc.tile_pool(name="const", bufs=1) as cpool,
            tc.tile_pool(name="s", bufs=2) as spool,
            tc.tile_pool(name="path", bufs=2) as ppool,
            tc.tile_pool(name="hr", bufs=3) as hrpool,
            tc.tile_pool(name="hw", bufs=8) as hwpool,
            tc.tile_pool(name="ot", bufs=2) as otpool,
            tc.tile_pool(name="ph", bufs=2, space="PSUM") as phpool,
            tc.tile_pool(name="po", bufs=3, space="PSUM") as popool,
        ):
            xT = cpool.tile([D, BC], f16)
            wrexp = cpool.tile([D, 384], f16)
            w1p = cpool.tile([D, 4096], f16)
            w2p = cpool.tile([D, NCH * O], f16)
            nc.sync.dma_start(wrexp[:], wrexp_d[:])
            nc.sync.dma_start(xT[:, 0:NBW], xT_d[:, 0:NBW])
            for q in range(4):
                nc.sync.dma_start(w1p[:, q * 1024:(q + 1) * 1024],
                                  w1p_d[:, q * 1024:(q + 1) * 1024])
            nc.sync.dma_start(w2p[:], w2p_d[:])
            for q in range(1, NB):
                nc.sync.dma_start(xT[:, q * NBW:(q + 1) * NBW],
                                  xT_d[:, q * NBW:(q + 1) * NBW])
            if not b2_zero:
                b2s = cpool.tile([H, O], f16)
                nc.sync.dma_start(b2s[:], b2s_d[:])
            if not br_zero:
                brexp = cpool.tile([D, 3], f32)
                nc.sync.dma_start(brexp[:], brexp_d[:])
            if not b1_zero:
                b1p = cpool.tile([D, NCH], f32)
                nc.sync.dma_start(b1p[:], b1p_d[:])

            def emit_router(nb):
                """G0 + sigmoid + path products for batch chunk nb.
                Returns (w, path2): path^T [64, NBW] and its 2x partition
                replica [128, NBW], both fp16."""
                bs = slice(nb * NBW, (nb + 1) * NBW)
                s = spool.tile([D, 3 * NBW], f16)
                for k in range(3):
                    pz = popool.tile([D, NBW], f32, tag="po")
                    nc.tensor.matmul(
                        pz[:],
                        wrexp[:, k * D:(k + 1) * D],
                        xT[:, bs],
                        start=True, stop=True,
                    )
                    nc.scalar.activation(
                        s[:, k * NBW:(k + 1) * NBW], pz[:], Act.Sigmoid,
                        **({} if br_zero else {"bias": brexp[:, k:k + 1]}),
                    )
                # s chunk layout (host WrExp col order):
                # chunk0 = [lvl0 | lvl3], chunk1 = [lvl1 | lvl4], chunk2 = [lvl2 | lvl5]
                # DVE operands must share a start partition, so the cross-half
                # combine goes through SBUF->SBUF DMA.
                c0 = s[:, 0:NBW]
                c1 = s[:, NBW:2 * NBW]
                c2 = s[:, 2 * NBW:3 * NBW]
                t1 = ppool.tile([D, NBW], f16, tag="t1")
                nc.gpsimd.tensor_tensor(t1[:], c0, c1, Alu.mult)     # [l01 | l34]
                t2 = ppool.tile([D, NBW], f16, tag="t2")
                nc.gpsimd.tensor_tensor(t2[:], t1[:], c2, Alu.mult)  # [l012 | l345]
                u = ppool.tile([H, NBW], f16, tag="u")
                nc.sync.dma_start(u[:], t2[H:2 * H, :])
                w = ppool.tile([H, NBW], f16, tag="w")
                nc.gpsimd.tensor_tensor(w[:], t2[0:H, :], u[:], Alu.mult)  # path^T
                path2 = ppool.tile([D, NBW], f16, tag="path2")
                nc.sync.dma_start(path2[0:H, :], w[:])
                nc.sync.dma_start(path2[H:2 * H, :], w[:])
                return w, path2

            def emit_casts(pending):
                # partition-aligned PSUM->SBUF copies (fp16), 4 DMAs out;
                # split between ACT and DVE for engine balance
                ppo, pnb = pending
                pbs = slice(pnb * NBW, (pnb + 1) * NBW)
                ot = otpool.tile([D, NBW], f16)
                for g in range(4):
                    if g < 2:
                        nc.scalar.activation(
                            ot[32 * g:32 * g + O, :],
                            ppo[32 * g:32 * g + O, :], Act.Copy)
                    else:
                        nc.vector.tensor_copy(
                            ot[32 * g:32 * g + O, :],
                            ppo[32 * g:32 * g + O, :])
                for g in range(4):
                    nc.sync.dma_start(
                        outT_d[O * g:O * (g + 1), pbs],
                        ot[32 * g:32 * g + O, :],
                    )

            pending = None
            cur = emit_router(0)
            for nb in range(NB):
                bs = slice(nb * NBW, (nb + 1) * NBW)
                w, path2 = cur

                # G2 is interleaved into the cg loop, delayed by G2_LAG units,
                # so PE never queues behind the full evacuation.
                G2_LAG = 2
                po = popool.tile([D, NBW], f32, tag="po")

                def emit_g2_unit(cgu):
                    for c in (2 * cgu, 2 * cgu + 1):
                        g = c % 4
                        last = c >= NCH - 4 and (b2_zero or g != 0)
                        nc.tensor.matmul(
                            po[32 * g:32 * g + O, :],
                            w2p[:, O * c:O * (c + 1)],
                            hw_tiles[c // 2][:, (c & 1) * NBW:((c & 1) + 1) * NBW],
                            start=(c < 4),
                            stop=last,
                            tile_position=(0, 32 * g),
                        )

                # G1 (+ fused evacuation) over 16 cg units = 32 k' chunks
                hw_tiles = []
                for cg in range(16):
                    ph = phpool.tile([D, 2 * NBW], f32)
                    for half in range(2):
                        c = 2 * cg + half
                        nc.tensor.matmul(
                            ph[:, half * NBW:(half + 1) * NBW],
                            w1p[:, c * D:(c + 1) * D],
                            xT[:, bs],
                            start=True, stop=True,
                        )
                    if cg == 3 and nb + 1 < NB:
                        nxt = emit_router(nb + 1)
                    if cg == 5 and pending is not None:
                        emit_casts(pending)
                        pending = None
                    hw = hwpool.tile([D, 2 * NBW], f16)
                    hw_tiles.append(hw)
                    # [D, 2, NBW] views so path2 broadcasts along the pair dim
                    path2b = path2[:].rearrange(
                        "p (one f) -> p one f", one=1).broadcast_to([D, 2, NBW])
                    hw3 = hw[:].rearrange("p (a f) -> p a f", a=2)
                    ph3 = ph[:].rearrange("p (a f) -> p a f", a=2)
                    if b1_zero and cg in STT_SET:
                        # DVE route: hw = (h max 0) * path2, straight from PSUM
                        nc.vector.scalar_tensor_tensor(
                            hw3, ph3, 0.0, path2b, Alu.max, Alu.mult,
                        )
                    else:
                        # ACT route: relu(+b1) cast to fp16, then DVE multiply
                        hr = hrpool.tile([D, 2 * NBW], f16)
                        if b1_zero:
                            nc.scalar.activation(hr[:], ph[:], Act.Relu)
                        else:
                            for half in range(2):
                                c = 2 * cg + half
                                nc.scalar.activation(
                                    hr[:, half * NBW:(half + 1) * NBW],
                                    ph[:, half * NBW:(half + 1) * NBW],
                                    Act.Relu, bias=b1p[:, c:c + 1],
                                )
                        hr3 = hr[:].rearrange("p (a f) -> p a f", a=2)
                        nc.vector.tensor_tensor(hw3, hr3, path2b, Alu.mult)
                    if cg >= G2_LAG:
                        emit_g2_unit(cg - G2_LAG)
                for cgu in range(16 - G2_LAG, 16):
                    emit_g2_unit(cgu)
                if not b2_zero:
                    nc.tensor.matmul(
                        po[0:O, :], b2s[:], w[:],
                        start=False, stop=True, tile_position=(0, 0),
                    )

                pending = (po, nb)
                if nb + 1 < NB:
                    cur = nxt
            emit_casts(pending)

    nc.compile()
    return nc


def _prep(x, Wr, br, W1, b1, W2, b2):
    """Host-side operand preparation (all fp16 except biases)."""
    chunk_levels = [(0, 3), (1, 4), (2, 5)]
    WrExp = np.zeros((D, 384), np.float32)
    brExp = np.zeros(384, np.float32)
    for k, levels in enumerate(chunk_levels):
        for half, lev in enumerate(levels):
            for l in range(L):
                q = 128 * k + 64 * half + l
                node = (2 ** lev - 1) + (l >> (DEPTH - lev))
                bit = (l >> (DEPTH - 1 - lev)) & 1
                sign = 1.0 - 2.0 * bit
                WrExp[:, q] = sign * Wr[node, :]
                brExp[q] = sign * br[node]

    W1p = W1.transpose(1, 2, 0).reshape(D, 4096)       # [d, j*64+l]
    W2p = W2.transpose(1, 0, 2).reshape(4096, O)       # [j*64+l, o]
    W2sb = W2p.reshape(NCH, D, O).transpose(1, 0, 2).reshape(D, NCH * O)
    b1p = b1.T.reshape(4096)                           # [j*64+l]

    shared = {
        "WrExp": np.ascontiguousarray(WrExp.astype(np.float16)),
        "W1p": np.ascontiguousarray(W1p.astype(np.float16)),
        "W2p": np.ascontiguousarray(W2sb.astype(np.float16)),
    }
    flags = (not np.any(brExp), not np.any(b1p), not np.any(b2))
    br_zero, b1_zero, b2_zero = flags
    if not b2_zero:
        shared["b2s"] = np.ascontiguousarray(b2.astype(np.float16))
    if not br_zero:
        shared["brExp"] = np.ascontiguousarray(brExp.reshape(3, D).T.astype(np.float32))
    if not b1_zero:
        shared["b1p"] = np.ascontiguousarray(b1p.reshape(NCH, D).T.astype(np.float32))
    return shared, flags


def kernel(x, Wr, br, W1, b1, W2, b2):
    from concourse.bass_utils import run_bass_kernel_spmd

    x = np.asarray(x, np.float32)
    Wr = np.asarray(Wr, np.float32)
    br = np.asarray(br, np.float32)
    W1 = np.asarray(W1, np.float32)
    b1 = np.asarray(b1, np.float32)
    W2 = np.asarray(W2, np.float32)
    b2 = np.asarray(b2, np.float32)

    shared, flags = _prep(x, Wr, br, W1, b1, W2, b2)
    if flags not in _cache:
        _cache[flags] = _build_module(*flags)
    nc = _cache[flags]

    in_maps = []
    for i in range(N_CORES):
        m = dict(shared)
        m["xT"] = np.ascontiguousarray(
            x[i * BC:(i + 1) * BC].T.astype(np.float16)
        )
        in_maps.append(m)

    res = run_bass_kernel_spmd(nc, in_maps, list(range(N_CORES)))
    out = np.empty((B, O), np.float32)
    for i in range(N_CORES):
        p4 = res.results[i]["outT4"].astype(np.float32)
        out[i * BC:(i + 1) * BC] = p4.reshape(4, O, BC).sum(axis=0).T
    return out


# revision 25
# speedup vs baseline: 1.1410x; 1.0949x over previous
"""Trainium2 Bass kernel for the soft neural decision tree (moe_routing).

Math (per batch row b):
  z~[q]   = sign(q) * (Wr[node(q)] . x[b])            q = (level, leaf) expanded
  path[l] = prod_level sigma(z~)                      (1 - sigma(z) = sigma(-z))
  h       = relu(x @ W1cat)                           W1cat in (j, l) interleaved order
  hw      = h * path[l(k')]                           broadcast over j
  out     = hw @ W2cat + path @ b2

Sharding: data-parallel over batch, 8 cores x 4096 rows. All GEMM inputs are
fp16 (1 cycle/row on the PE; fp32 is 4). The router/path stage for chunk
nb+1 is emitted mid-chunk so its latency chain hides under GEMM work.
PSUM evacuation of h is split between ScalarE (fused relu+cast) and VectorE
(fused scalar_tensor_tensor (h max 0) * path) to balance engines; the path
multiply runs on VectorE as fp16 tensor_tensor at 2x with a 0-stride
broadcast AP for the path operand. GEMM2 is k-split over 4 PE column-tile
groups whose partial sums are combined on the host.
"""

import numpy as np

N_CORES = 8
B = 32768
BC = B // N_CORES          # batch rows per core
NB = 8                     # batch sub-chunks per core
NBW = BC // NB             # 1024 columns per sub-chunk
D = 128
H = 64
L = 64
O = 10
DEPTH = 6
NCH = 32                   # k' chunks of 128 rows
N_STT = 5                  # cg units (of 16 per nb) evacuated via DVE STT route
# spread the DVE-routed units through the loop so ACT and DVE overlap
STT_SET = {(i * 16) // N_STT + 1 for i in range(N_STT)}

_cache = {}


def _build_module(br_zero, b1_zero, b2_zero):
    from concourse import bacc, tile
    import concourse.mybir as mybir

    f16 = mybir.dt.float16
    f32 = mybir.dt.float32
    Alu = mybir.AluOpType
    Act = mybir.ActivationFunctionType

    nc = bacc.Bacc(None)
    xT_d = nc.declare_dram_parameter("xT", [D, BC], f16, isOutput=False)
    wrexp_d = nc.declare_dram_parameter("WrExp", [D, 384], f16, isOutput=False)
    w1p_d = nc.declare_dram_parameter("W1p", [D, 4096], f16, isOutput=False)
    w2p_d = nc.declare_dram_parameter("W2p", [D, NCH * O], f16, isOutput=False)
    if not b2_zero:
        b2s_d = nc.declare_dram_parameter("b2s", [H, O], f16, isOutput=False)
    if not br_zero:
        brexp_d = nc.declare_dram_parameter("brExp", [D, 3], f32, isOutput=False)
    if not b1_zero:
        b1p_d = nc.declare_dram_parameter("b1p", [D, NCH], f32, isOutput=False)
    # 4 col-tiling partial sums per output element; summed on the host
    outT_d = nc.declare_dram_parameter("outT4", [4 * O, BC], f16, isOutput=True)

    with tile.TileContext(nc) as tc:
        with (
            tc.tile_pool(name="const", bufs=1) as cpool,
            tc.tile_pool(name="s", bufs=2) as spool,
            tc.tile_pool(name="path", bufs=2) as ppool,
            tc.tile_pool(name="hr", bufs=3) as hrpool,
            tc.tile_pool(name="hw", bufs=8) as hwpool,
            tc.tile_pool(name="ot", bufs=2) as otpool,
            tc.tile_pool(name="ph", bufs=3, space="PSUM") as phpool,
            tc.tile_pool(name="po", bufs=2, space="PSUM") as popool,
        ):
            xT = cpool.tile([D, BC], f16)
            wrexp = cpool.tile([D, 384], f16)
            w1p = cpool.tile([D, 4096], f16)
            w2p = cpool.tile([D, NCH * O], f16)
            nc.sync.dma_start(wrexp[:], wrexp_d[:])
            nc.sync.dma_start(xT[:, 0:NBW], xT_d[:, 0:NBW])
            for q in range(4):
                nc.sync.dma_start(w1p[:, q * 1024:(q + 1) * 1024],
                                  w1p_d[:, q * 1024:(q + 1) * 1024])
            nc.sync.dma_start(w2p[:], w2p_d[:])
            for q in range(1, NB):
                nc.sync.dma_start(xT[:, q * NBW:(q + 1) * NBW],
                                  xT_d[:, q * NBW:(q + 1) * NBW])
            if not b2_zero:
                b2s = cpool.tile([H, O], f16)
                nc.sync.dma_start(b2s[:], b2s_d[:])
            if not br_zero:
                brexp = cpool.tile([D, 3], f32)
                nc.sync.dma_start(brexp[:], brexp_d[:])
            if not b1_zero:
                b1p = cpool.tile([D, NCH], f32)
                nc.sync.dma_start(b1p[:], b1p_d[:])

            def emit_router(nb, use_dve=False):
                """G0 + sigmoid + path products for batch chunk nb.
                Returns (w, path2): path^T [64, NBW] and its 2x partition
                replica [128, NBW], both fp16."""
                bs = slice(nb * NBW, (nb + 1) * NBW)
                s = spool.tile([D, 3 * NBW], f16)
                for k in range(3):
                    pz = popool.tile([D, NBW], f32, tag="po")
                    nc.tensor.matmul(
                        pz[:],
                        wrexp[:, k * D:(k + 1) * D],
                        xT[:, bs],
                        start=True, stop=True,
                    )
                    nc.scalar.activation(
                        s[:, k * NBW:(k + 1) * NBW], pz[:], Act.Sigmoid,
                        **({} if br_zero else {"bias": brexp[:, k:k + 1]}),
                    )
                # s chunk layout (host WrExp col order):
                # chunk0 = [lvl0 | lvl3], chunk1 = [lvl1 | lvl4], chunk2 = [lvl2 | lvl5]
                # DVE operands must share a start partition, so the cross-half
                # combine goes through SBUF->SBUF DMA.
                c0 = s[:, 0:NBW]
                c1 = s[:, NBW:2 * NBW]
                c2 = s[:, 2 * NBW:3 * NBW]
                eng = nc.vector if use_dve else nc.gpsimd
                t1 = ppool.tile([D, NBW], f16, tag="t1")
                eng.tensor_tensor(t1[:], c0, c1, Alu.mult)     # [l01 | l34]
                t2 = ppool.tile([D, NBW], f16, tag="t2")
                eng.tensor_tensor(t2[:], t1[:], c2, Alu.mult)  # [l012 | l345]
                u = ppool.tile([H, NBW], f16, tag="u")
                nc.sync.dma_start(u[:], t2[H:2 * H, :])
                w = ppool.tile([H, NBW], f16, tag="w")
                eng.tensor_tensor(w[:], t2[0:H, :], u[:], Alu.mult)  # path^T
                path2 = ppool.tile([D, NBW], f16, tag="path2")
                nc.sync.dma_start(path2[0:H, :], w[:])
                nc.sync.dma_start(path2[H:2 * H, :], w[:])
                return w, path2

            def emit_casts(pending):
                # partition-aligned PSUM->SBUF copies (fp16), 4 DMAs out;
                # split between ACT and DVE for engine balance
                ppo, pnb = pending
                pbs = slice(pnb * NBW, (pnb + 1) * NBW)
                ot = otpool.tile([D, NBW], f16)
                for g in range(4):
                    if g < 2:
                        nc.scalar.activation(
                            ot[32 * g:32 * g + O, :],
                            ppo[32 * g:32 * g + O, :], Act.Copy)
                    else:
                        nc.vector.tensor_copy(
                            ot[32 * g:32 * g + O, :],
                            ppo[32 * g:32 * g + O, :])
                for g in range(4):
                    nc.sync.dma_start(
                        outT_d[O * g:O * (g + 1), pbs],
                        ot[32 * g:32 * g + O, :],
                    )

            pending = None
            cur = emit_router(0, use_dve=True)
            for nb in range(NB):
                bs = slice(nb * NBW, (nb + 1) * NBW)
                w, path2 = cur

                # G2 is interleaved into the cg loop, delayed by G2_LAG units,
                # so PE never queues behind the full evacuation.
                G2_LAG = 2
                po = popool.tile([D, NBW], f32, tag="po")

                def emit_g2_unit(cgu):
                    for c in (2 * cgu, 2 * cgu + 1):
                        g = c % 4
                        last = c >= NCH - 4 and (b2_zero or g != 0)
                        hwt, hoff = hw_of[c]
                        nc.tensor.matmul(
                            po[32 * g:32 * g + O, :],
                            w2p[:, O * c:O * (c + 1)],
                            hwt[:, hoff:hoff + NBW],
                            start=(c < 4),
                            stop=last,
                            tile_position=(0, 32 * g),
                        )

                # G1 (+ fused evacuation) over 16 cg units = 32 k' chunks.
                # ACT-route units are paired so the DVE path-multiply runs one
                # FD=2048 tensor_tensor per two units. G2 units are emitted
                # from a queue once their hw producer is emitted.
                hw_of = {}            # chunk c -> (tile, col offset)
                mult_done = set()     # cg units whose hw writes are emitted
                act_accum = None      # (hr2, hw2, [cg...])
                next_g2 = 0

                def flush_pair():
                    nonlocal act_accum
                    if act_accum is None:
                        return
                    hr2, hw2, cgs = act_accum
                    a = 2 * len(cgs)
                    p2b = path2[:].rearrange(
                        "p (one f) -> p one f", one=1).broadcast_to([D, a, NBW])
                    hr3 = hr2[:, 0:a * NBW].rearrange("p (a f) -> p a f", a=a)
                    hw3 = hw2[:, 0:a * NBW].rearrange("p (a f) -> p a f", a=a)
                    nc.vector.tensor_tensor(hw3, hr3, p2b, Alu.mult)
                    mult_done.update(cgs)
                    act_accum = None

                for cg in range(16):
                    ph = phpool.tile([D, 2 * NBW], f32)
                    for half in range(2):
                        c = 2 * cg + half
                        nc.tensor.matmul(
                            ph[:, half * NBW:(half + 1) * NBW],
                            w1p[:, c * D:(c + 1) * D],
                            xT[:, bs],
                            start=True, stop=True,
                        )
                    if cg == 3 and nb + 1 < NB:
                        nxt = emit_router(nb + 1)
                    if cg == 5 and pending is not None:
                        emit_casts(pending)
                        pending = None
                    if b1_zero and cg in STT_SET:
                        hw = hwpool.tile([D, 2 * NBW], f16, tag="hw")
                        p2b = path2[:].rearrange(
                            "p (one f) -> p one f", one=1).broadcast_to([D, 2, NBW])
                        hw3 = hw[:].rearrange("p (a f) -> p a f", a=2)
                        ph3 = ph[:].rearrange("p (a f) -> p a f", a=2)
                        nc.vector.scalar_tensor_tensor(
                            hw3, ph3, 0.0, p2b, Alu.max, Alu.mult,
                        )
                        hw_of[2 * cg] = (hw, 0)
                        hw_of[2 * cg + 1] = (hw, NBW)
                        mult_done.add(cg)
                    else:
                        if act_accum is None:
                            hr2 = hrpool.tile([D, 4 * NBW], f16, tag="hr")
                            hw2 = hwpool.tile([D, 4 * NBW], f16, tag="hw2")
                            act_accum = (hr2, hw2, [])
                        hr2, hw2, cgs = act_accum
                        off = len(cgs) * 2 * NBW
                        if b1_zero:
                            nc.scalar.activation(
                                hr2[:, off:off + 2 * NBW], ph[:], Act.Relu)
                        else:
                            for half in range(2):
                                c = 2 * cg + half
                                nc.scalar.activation(
                                    hr2[:, off + half * NBW:off + (half + 1) * NBW],
                                    ph[:, half * NBW:(half + 1) * NBW],
                                    Act.Relu, bias=b1p[:, c:c + 1],
                                )
                        hw_of[2 * cg] = (hw2, off)
                        hw_of[2 * cg + 1] = (hw2, off + NBW)
                        cgs.append(cg)
                        if len(cgs) == 2:
                            flush_pair()
                    while next_g2 <= cg - G2_LAG and next_g2 in mult_done:
                        emit_g2_unit(next_g2)
                        next_g2 += 1
                flush_pair()
                while next_g2 < 16:
                    emit_g2_unit(next_g2)
                    next_g2 += 1
                if not b2_zero:
                    nc.tensor.matmul(
                        po[0:O, :], b2s[:], w[:],
                        start=False, stop=True, tile_position=(0, 0),
                    )

                pending = (po, nb)
                if nb + 1 < NB:
                    cur = nxt
            emit_casts(pending)

    nc.compile()
    return nc


def _prep(x, Wr, br, W1, b1, W2, b2):
    """Host-side operand preparation (all fp16 except biases)."""
    chunk_levels = [(0, 3), (1, 4), (2, 5)]
    WrExp = np.zeros((D, 384), np.float32)
    brExp = np.zeros(384, np.float32)
    for k, levels in enumerate(chunk_levels):
        for half, lev in enumerate(levels):
            for l in range(L):
                q = 128 * k + 64 * half + l
                node = (2 ** lev - 1) + (l >> (DEPTH - lev))
                bit = (l >> (DEPTH - 1 - lev)) & 1
                sign = 1.0 - 2.0 * bit
                WrExp[:, q] = sign * Wr[node, :]
                brExp[q] = sign * br[node]

    W1p = W1.transpose(1, 2, 0).reshape(D, 4096)       # [d, j*64+l]
    W2p = W2.transpose(1, 0, 2).reshape(4096, O)       # [j*64+l, o]
    W2sb = W2p.reshape(NCH, D, O).transpose(1, 0, 2).reshape(D, NCH * O)
    b1p = b1.T.reshape(4096)                           # [j*64+l]

    shared = {
        "WrExp": np.ascontiguousarray(WrExp.astype(np.float16)),
        "W1p": np.ascontiguousarray(W1p.astype(np.float16)),
        "W2p": np.ascontiguousarray(W2sb.astype(np.float16)),
    }
    flags = (not np.any(brExp), not np.any(b1p), not np.any(b2))
    br_zero, b1_zero, b2_zero = flags
    if not b2_zero:
        shared["b2s"] = np.ascontiguousarray(b2.astype(np.float16))
    if not br_zero:
        shared["brExp"] = np.ascontiguousarray(brExp.reshape(3, D).T.astype(np.float32))
    if not b1_zero:
        shared["b1p"] = np.ascontiguousarray(b1p.reshape(NCH, D).T.astype(np.float32))
    return shared, flags


def kernel(x, Wr, br, W1, b1, W2, b2):
    from concourse.bass_utils import run_bass_kernel_spmd

    x = np.asarray(x, np.float32)
    Wr = np.asarray(Wr, np.float32)
    br = np.asarray(br, np.float32)
    W1 = np.asarray(W1, np.float32)
    b1 = np.asarray(b1, np.float32)
    W2 = np.asarray(W2, np.float32)
    b2 = np.asarray(b2, np.float32)

    shared, flags = _prep(x, Wr, br, W1, b1, W2, b2)
    if flags not in _cache:
        _cache[flags] = _build_module(*flags)
    nc = _cache[flags]

    in_maps = []
    for i in range(N_CORES):
        m = dict(shared)
        m["xT"] = np.ascontiguousarray(
            x[i * BC:(i + 1) * BC].T.astype(np.float16)
        )
        in_maps.append(m)

    res = run_bass_kernel_spmd(nc, in_maps, list(range(N_CORES)))
    out = np.empty((B, O), np.float32)
    for i in range(N_CORES):
        p4 = res.results[i]["outT4"].astype(np.float32)
        out[i * BC:(i + 1) * BC] = p4.reshape(4, O, BC).sum(axis=0).T
    return out


# revision 29
# speedup vs baseline: 1.1804x; 1.0345x over previous
"""Trainium2 Bass kernel for the soft neural decision tree (moe_routing).

Math (per batch row b):
  z~[q]   = sign(q) * (Wr[node(q)] . x[b])            q = (level, leaf) expanded
  path[l] = prod_level sigma(z~)                      (1 - sigma(z) = sigma(-z))
  h       = relu(x @ W1cat)                           W1cat in (j, l) interleaved order
  hw      = h * path[l(k')]                           broadcast over j
  out     = hw @ W2cat + path @ b2

Sharding: data-parallel over batch, 8 cores x 4096 rows. All GEMM inputs are
fp16 (1 cycle/row on the PE; fp32 is 4). The router/path stage for chunk
nb+1 is emitted mid-chunk so its latency chain hides under GEMM work.
PSUM evacuation of h is split between ScalarE (fused relu+cast) and VectorE
(fused scalar_tensor_tensor (h max 0) * path) to balance engines; the path
multiply runs on VectorE as fp16 tensor_tensor at 2x with a 0-stride
broadcast AP for the path operand. GEMM2 is k-split over 4 PE column-tile
groups whose partial sums are combined on the host.
"""

import numpy as np

N_CORES = 8
B = 32768
BC = B // N_CORES          # batch rows per core
NB = 8                     # batch sub-chunks per core
NBW = BC // NB             # 1024 columns per sub-chunk
D = 128
H = 64
L = 64
O = 10
DEPTH = 6
NCH = 32                   # k' chunks of 128 rows
N_STT = 5                  # cg units (of 16 per nb) evacuated via DVE STT route
# spread the DVE-routed units through the loop so ACT and DVE overlap
STT_SET = {(i * 16) // N_STT + 1 for i in range(N_STT)}

_cache = {}


def _build_module(br_zero, b1_zero, b2_zero):
    from concourse import bacc, tile
    import concourse.mybir as mybir

    f16 = mybir.dt.float16
    f32 = mybir.dt.float32
    Alu = mybir.AluOpType
    Act = mybir.ActivationFunctionType

    nc = bacc.Bacc(None)
    xT_d = nc.declare_dram_parameter("xT", [D, BC], f16, isOutput=False)
    wrexp_d = nc.declare_dram_parameter("WrExp", [D, 384], f16, isOutput=False)
    w1p_d = nc.declare_dram_parameter("W1p", [D, 4096], f16, isOutput=False)
    w2p_d = nc.declare_dram_parameter("W2p", [D, NCH * O], f16, isOutput=False)
    if not b2_zero:
        b2s_d = nc.declare_dram_parameter("b2s", [H, O], f16, isOutput=False)
    if not br_zero:
        brexp_d = nc.declare_dram_parameter("brExp", [D, 3], f32, isOutput=False)
    if not b1_zero:
        b1p_d = nc.declare_dram_parameter("b1p", [D, NCH], f32, isOutput=False)
    outT_d = nc.declare_dram_parameter("outT", [O, BC], f16, isOutput=True)

    with tile.TileContext(nc) as tc:
        with (
            tc.tile_pool(name="const", bufs=1) as cpool,
            tc.tile_pool(name="s", bufs=2) as spool,
            tc.tile_pool(name="path", bufs=2) as ppool,
            tc.tile_pool(name="hr", bufs=3) as hrpool,
            tc.tile_pool(name="hw", bufs=8) as hwpool,
            tc.tile_pool(name="ot", bufs=2) as otpool,
            tc.tile_pool(name="ph", bufs=3, space="PSUM") as phpool,
            tc.tile_pool(name="po", bufs=2, space="PSUM") as popool,
        ):
            xT = cpool.tile([D, BC], f16)
            wrexp = cpool.tile([D, 384], f16)
            w1p = cpool.tile([D, 4096], f16)
            w2p = cpool.tile([D, NCH * O], f16)
            nc.sync.dma_start(xT[:, 0:NBW], xT_d[:, 0:NBW])
            nc.gpsimd.dma_start(w1p[:, 0:1024], w1p_d[:, 0:1024])
            nc.sync.dma_start(wrexp[:], wrexp_d[:])
            for q in range(1, 4):
                nc.gpsimd.dma_start(w1p[:, q * 1024:(q + 1) * 1024],
                                    w1p_d[:, q * 1024:(q + 1) * 1024])
            nc.sync.dma_start(w2p[:], w2p_d[:])
            for q in range(1, NB):
                (nc.sync if q % 2 else nc.gpsimd).dma_start(
                    xT[:, q * NBW:(q + 1) * NBW],
                    xT_d[:, q * NBW:(q + 1) * NBW])
            if not b2_zero:
                b2s = cpool.tile([H, O], f16)
                nc.sync.dma_start(b2s[:], b2s_d[:])
            if not br_zero:
                brexp = cpool.tile([D, 3], f32)
                nc.sync.dma_start(brexp[:], brexp_d[:])
            if not b1_zero:
                b1p = cpool.tile([D, NCH], f32)
                nc.sync.dma_start(b1p[:], b1p_d[:])

            def emit_router(nb, use_dve=False):
                """G0 + sigmoid + path products for batch chunk nb.
                Returns (w, path2): path^T [64, NBW] and its 2x partition
                replica [128, NBW], both fp16."""
                bs = slice(nb * NBW, (nb + 1) * NBW)
                s = spool.tile([D, 3 * NBW], f16)
                pz2 = phpool.tile([D, 2 * NBW], f32, tag="ph")
                for k in range(2):
                    nc.tensor.matmul(
                        pz2[:, k * NBW:(k + 1) * NBW],
                        wrexp[:, k * D:(k + 1) * D],
                        xT[:, bs],
                        start=True, stop=True,
                    )
                if br_zero:
                    nc.scalar.activation(s[:, 0:2 * NBW], pz2[:], Act.Sigmoid)
                else:
                    for k in range(2):
                        nc.scalar.activation(
                            s[:, k * NBW:(k + 1) * NBW],
                            pz2[:, k * NBW:(k + 1) * NBW],
                            Act.Sigmoid, bias=brexp[:, k:k + 1],
                        )
                pz = popool.tile([D, NBW], f32, tag="po")
                nc.tensor.matmul(
                    pz[:], wrexp[:, 2 * D:3 * D], xT[:, bs],
                    start=True, stop=True,
                )
                nc.scalar.activation(
                    s[:, 2 * NBW:3 * NBW], pz[:], Act.Sigmoid,
                    **({} if br_zero else {"bias": brexp[:, 2:3]}),
                )
                # s chunk layout (host WrExp col order):
                # chunk0 = [lvl0 | lvl3], chunk1 = [lvl1 | lvl4], chunk2 = [lvl2 | lvl5]
                # DVE operands must share a start partition, so the cross-half
                # combine goes through SBUF->SBUF DMA.
                c0 = s[:, 0:NBW]
                c1 = s[:, NBW:2 * NBW]
                c2 = s[:, 2 * NBW:3 * NBW]
                eng = nc.vector if use_dve else nc.gpsimd
                t1 = ppool.tile([D, NBW], f16, tag="t1")
                eng.tensor_tensor(t1[:], c0, c1, Alu.mult)     # [l01 | l34]
                t2 = ppool.tile([D, NBW], f16, tag="t2")
                eng.tensor_tensor(t2[:], t1[:], c2, Alu.mult)  # [l012 | l345]
                u = ppool.tile([H, NBW], f16, tag="u")
                nc.sync.dma_start(u[:], t2[H:2 * H, :])
                w = ppool.tile([H, NBW], f16, tag="w")
                eng.tensor_tensor(w[:], t2[0:H, :], u[:], Alu.mult)  # path^T
                path2 = ppool.tile([D, NBW], f16, tag="path2")
                nc.sync.dma_start(path2[0:H, :], w[:])
                nc.sync.dma_start(path2[H:2 * H, :], w[:])
                return w, path2

            def emit_casts(pending):
                # single PSUM->SBUF cast (fp16) + one DMA out; alternate the
                # engine per chunk for balance
                ppo, pnb = pending
                pbs = slice(pnb * NBW, (pnb + 1) * NBW)
                ot = otpool.tile([O, NBW], f16)
                if pnb % 2 == 0:
                    nc.scalar.activation(ot[:], ppo[0:O, :], Act.Copy)
                else:
                    nc.vector.tensor_copy(ot[:], ppo[0:O, :])
                nc.sync.dma_start(outT_d[:, pbs], ot[:])

            # HAM warmup: keep the PE busy on scratch data while the input
            # DMAs land, so real matmuls start at 2.4 GHz
            scratch = cpool.tile([D, NBW], f16)
            nc.vector.memset(scratch[:], 0.0)
            pwarm = popool.tile([D, NBW], f32, tag="po")
            for _ in range(8):
                nc.tensor.matmul(pwarm[0:D, :], scratch[:, 0:D], scratch[:],
                                 start=True, stop=True)

            pending = None
            cur = emit_router(0, use_dve=True)
            for nb in range(NB):
                bs = slice(nb * NBW, (nb + 1) * NBW)
                w, path2 = cur

                # G2 is interleaved into the cg loop, delayed by G2_LAG units,
                # so PE never queues behind the full evacuation.
                G2_LAG = 2
                po = popool.tile([D, NBW], f32, tag="po")

                def emit_g2_unit(cgu):
                    for c in (2 * cgu, 2 * cgu + 1):
                        hwt, hoff = hw_of[c]
                        nc.tensor.matmul(
                            po[0:O, :],
                            w2p[:, O * c:O * (c + 1)],
                            hwt[:, hoff:hoff + NBW],
                            start=(c == 0),
                            stop=(c == NCH - 1 and b2_zero),
                        )

                # G1 (+ fused evacuation) over 16 cg units = 32 k' chunks.
                # ACT-route units are paired so the DVE path-multiply runs one
                # FD=2048 tensor_tensor per two units. G2 units are emitted
                # from a queue once their hw producer is emitted.
                hw_of = {}            # chunk c -> (tile, col offset)
                mult_done = set()     # cg units whose hw writes are emitted
                act_accum = None      # (hr2, hw2, [cg...])
                next_g2 = 0

                def flush_pair():
                    nonlocal act_accum
                    if act_accum is None:
                        return
                    hr2, hw2, cgs = act_accum
                    a = 2 * len(cgs)
                    p2b = path2[:].rearrange(
                        "p (one f) -> p one f", one=1).broadcast_to([D, a, NBW])
                    hr3 = hr2[:, 0:a * NBW].rearrange("p (a f) -> p a f", a=a)
                    hw3 = hw2[:, 0:a * NBW].rearrange("p (a f) -> p a f", a=a)
                    nc.vector.tensor_tensor(hw3, hr3, p2b, Alu.mult)
                    mult_done.update(cgs)
                    act_accum = None

                for cg in range(16):
                    ph = phpool.tile([D, 2 * NBW], f32, tag="ph")
                    for half in range(2):
                        c = 2 * cg + half
                        nc.tensor.matmul(
                            ph[:, half * NBW:(half + 1) * NBW],
                            w1p[:, c * D:(c + 1) * D],
                            xT[:, bs],
                            start=True, stop=True,
                        )
                    if cg == 3 and nb + 1 < NB:
                        nxt = emit_router(nb + 1)
                    if cg == 1 and pending is not None:
                        emit_casts(pending)
                        pending = None
                    if b1_zero and cg in STT_SET:
                        hw = hwpool.tile([D, 2 * NBW], f16, tag="hw")
                        p2b = path2[:].rearrange(
                            "p (one f) -> p one f", one=1).broadcast_to([D, 2, NBW])
                        hw3 = hw[:].rearrange("p (a f) -> p a f", a=2)
                        ph3 = ph[:].rearrange("p (a f) -> p a f", a=2)
                        nc.vector.scalar_tensor_tensor(
                            hw3, ph3, 0.0, p2b, Alu.max, Alu.mult,
                        )
                        hw_of[2 * cg] = (hw, 0)
                        hw_of[2 * cg + 1] = (hw, NBW)
                        mult_done.add(cg)
                    else:
                        if act_accum is None:
                            hr2 = hrpool.tile([D, 4 * NBW], f16, tag="hr")
                            hw2 = hwpool.tile([D, 4 * NBW], f16, tag="hw2")
                            act_accum = (hr2, hw2, [])
                        hr2, hw2, cgs = act_accum
                        off = len(cgs) * 2 * NBW
                        if b1_zero:
                            nc.scalar.activation(
                                hr2[:, off:off + 2 * NBW], ph[:], Act.Relu)
                        else:
                            for half in range(2):
                                c = 2 * cg + half
                                nc.scalar.activation(
                                    hr2[:, off + half * NBW:off + (half + 1) * NBW],
                                    ph[:, half * NBW:(half + 1) * NBW],
                                    Act.Relu, bias=b1p[:, c:c + 1],
                                )
                        hw_of[2 * cg] = (hw2, off)
                        hw_of[2 * cg + 1] = (hw2, off + NBW)
                        cgs.append(cg)
                        if len(cgs) == 2:
                            flush_pair()
                    while next_g2 <= cg - G2_LAG and next_g2 in mult_done:
                        emit_g2_unit(next_g2)
                        next_g2 += 1
                flush_pair()
                while next_g2 < 16:
                    emit_g2_unit(next_g2)
                    next_g2 += 1
                if not b2_zero:
                    nc.tensor.matmul(
                        po[0:O, :], b2s[:], w[:],
                        start=False, stop=True,
                    )

                pending = (po, nb)
                if nb + 1 < NB:
                    cur = nxt
            emit_casts(pending)

    nc.compile()
    return nc


def _prep(x, Wr, br, W1, b1, W2, b2):
    """Host-side operand preparation (all fp16 except biases)."""
    chunk_levels = [(0, 3), (1, 4), (2, 5)]
    WrExp = np.zeros((D, 384), np.float32)
    brExp = np.zeros(384, np.float32)
    for k, levels in enumerate(chunk_levels):
        for half, lev in enumerate(levels):
            for l in range(L):
                q = 128 * k + 64 * half + l
                node = (2 ** lev - 1) + (l >> (DEPTH - lev))
                bit = (l >> (DEPTH - 1 - lev)) & 1
                sign = 1.0 - 2.0 * bit
                WrExp[:, q] = sign * Wr[node, :]
                brExp[q] = sign * br[node]

    W1p = W1.transpose(1, 2, 0).reshape(D, 4096)       # [d, j*64+l]
    W2p = W2.transpose(1, 0, 2).reshape(4096, O)       # [j*64+l, o]
    W2sb = W2p.reshape(NCH, D, O).transpose(1, 0, 2).reshape(D, NCH * O)
    b1p = b1.T.reshape(4096)                           # [j*64+l]

    shared = {
        "WrExp": np.ascontiguousarray(WrExp.astype(np.float16)),
        "W1p": np.ascontiguousarray(W1p.astype(np.float16)),
        "W2p": np.ascontiguousarray(W2sb.astype(np.float16)),
    }
    flags = (not np.any(brExp), not np.any(b1p), not np.any(b2))
    br_zero, b1_zero, b2_zero = flags
    if not b2_zero:
        shared["b2s"] = np.ascontiguousarray(b2.astype(np.float16))
    if not br_zero:
        shared["brExp"] = np.ascontiguousarray(brExp.reshape(3, D).T.astype(np.float32))
    if not b1_zero:
        shared["b1p"] = np.ascontiguousarray(b1p.reshape(NCH, D).T.astype(np.float32))
    return shared, flags


def kernel(x, Wr, br, W1, b1, W2, b2):
    from concourse.bass_utils import run_bass_kernel_spmd

    x = np.asarray(x, np.float32)
    Wr = np.asarray(Wr, np.float32)
    br = np.asarray(br, np.float32)
    W1 = np.asarray(W1, np.float32)
    b1 = np.asarray(b1, np.float32)
    W2 = np.asarray(W2, np.float32)
    b2 = np.asarray(b2, np.float32)

    shared, flags = _prep(x, Wr, br, W1, b1, W2, b2)
    if flags not in _cache:
        _cache[flags] = _build_module(*flags)
    nc = _cache[flags]

    in_maps = []
    for i in range(N_CORES):
        m = dict(shared)
        m["xT"] = np.ascontiguousarray(
            x[i * BC:(i + 1) * BC].T.astype(np.float16)
        )
        in_maps.append(m)

    res = run_bass_kernel_spmd(nc, in_maps, list(range(N_CORES)))
    out = np.empty((B, O), np.float32)
    for i in range(N_CORES):
        out[i * BC:(i + 1) * BC] = res.results[i]["outT"].astype(np.float32).T
    return out


# revision 30
# speedup vs baseline: 1.2378x; 1.0486x over previous
"""Trainium2 Bass kernel for the soft neural decision tree (moe_routing).

Math (per batch row b):
  z~[q]   = sign(q) * (Wr[node(q)] . x[b])            q = (level, leaf) expanded
  path[l] = prod_level sigma(z~)                      (1 - sigma(z) = sigma(-z))
  h       = relu(x @ W1cat)                           W1cat in (j, l) interleaved order
  hw      = h * path[l(k')]                           broadcast over j
  out     = hw @ W2cat + path @ b2

Sharding: data-parallel over batch, 8 cores x 4096 rows. All GEMM inputs are
fp16 (1 cycle/row on the PE; fp32 is 4). The router/path stage for chunk
nb+1 is emitted mid-chunk so its latency chain hides under GEMM work.
PSUM evacuation of h is split between ScalarE (fused relu+cast) and VectorE
(fused scalar_tensor_tensor (h max 0) * path) to balance engines; the path
multiply runs on VectorE as fp16 tensor_tensor at 2x with a 0-stride
broadcast AP for the path operand. GEMM2 is k-split over 4 PE column-tile
groups whose partial sums are combined on the host.
"""

import numpy as np

N_CORES = 8
B = 32768
BC = B // N_CORES          # batch rows per core
NB = 8                     # batch sub-chunks per core
NBW = BC // NB             # 1024 columns per sub-chunk
D = 128
H = 64
L = 64
O = 10
DEPTH = 6
NCH = 32                   # k' chunks of 128 rows
N_STT = 5                  # cg units (of 16 per nb) evacuated via DVE STT route
# spread the DVE-routed units through the loop so ACT and DVE overlap
STT_SET = {(i * 16) // N_STT + 1 for i in range(N_STT)}

_cache = {}


def _build_module(br_zero, b1_zero, b2_zero):
    from concourse import bacc, tile
    import concourse.mybir as mybir

    f16 = mybir.dt.float16
    f32 = mybir.dt.float32
    Alu = mybir.AluOpType
    Act = mybir.ActivationFunctionType

    nc = bacc.Bacc(None)
    xT_d = nc.declare_dram_parameter("xT", [D, BC], f16, isOutput=False)
    wrexp_d = nc.declare_dram_parameter("WrExp", [D, 384], f16, isOutput=False)
    w1p_d = nc.declare_dram_parameter("W1p", [D, 4096], f16, isOutput=False)
    w2p_d = nc.declare_dram_parameter("W2p", [D, NCH * O], f16, isOutput=False)
    if not b2_zero:
        b2s_d = nc.declare_dram_parameter("b2s", [H, O], f16, isOutput=False)
    if not br_zero:
        brexp_d = nc.declare_dram_parameter("brExp", [D, 3], f32, isOutput=False)
    if not b1_zero:
        b1p_d = nc.declare_dram_parameter("b1p", [D, NCH], f32, isOutput=False)
    # 4 col-tiling partial sums per output element; summed on the host
    outT_d = nc.declare_dram_parameter("outT4", [4 * O, BC], f16, isOutput=True)

    with tile.TileContext(nc) as tc:
        with (
            tc.tile_pool(name="const", bufs=1) as cpool,
            tc.tile_pool(name="s", bufs=2) as spool,
            tc.tile_pool(name="path", bufs=2) as ppool,
            tc.tile_pool(name="hr", bufs=3) as hrpool,
            tc.tile_pool(name="hw", bufs=8) as hwpool,
            tc.tile_pool(name="ot", bufs=2) as otpool,
            tc.tile_pool(name="ph", bufs=3, space="PSUM") as phpool,
            tc.tile_pool(name="po", bufs=2, space="PSUM") as popool,
        ):
            xT = cpool.tile([D, BC], f16)
            wrexp = cpool.tile([D, 384], f16)
            w1p = cpool.tile([D, 4096], f16)
            w2p = cpool.tile([D, NCH * O], f16)
            nc.sync.dma_start(xT[:, 0:NBW], xT_d[:, 0:NBW])
            nc.gpsimd.dma_start(w1p[:, 0:1024], w1p_d[:, 0:1024])
            nc.sync.dma_start(wrexp[:], wrexp_d[:])
            for q in range(1, 4):
                nc.gpsimd.dma_start(w1p[:, q * 1024:(q + 1) * 1024],
                                    w1p_d[:, q * 1024:(q + 1) * 1024])
            nc.sync.dma_start(w2p[:], w2p_d[:])
            for q in range(1, NB):
                (nc.sync if q % 2 else nc.gpsimd).dma_start(
                    xT[:, q * NBW:(q + 1) * NBW],
                    xT_d[:, q * NBW:(q + 1) * NBW])
            if not b2_zero:
                b2s = cpool.tile([H, O], f16)
                nc.sync.dma_start(b2s[:], b2s_d[:])
            if not br_zero:
                brexp = cpool.tile([D, 3], f32)
                nc.sync.dma_start(brexp[:], brexp_d[:])
            if not b1_zero:
                b1p = cpool.tile([D, NCH], f32)
                nc.sync.dma_start(b1p[:], b1p_d[:])

            def emit_router(nb, use_dve=False):
                """G0 + sigmoid + path products for batch chunk nb.
                Returns (w, path2): path^T [64, NBW] and its 2x partition
                replica [128, NBW], both fp16."""
                bs = slice(nb * NBW, (nb + 1) * NBW)
                s = spool.tile([D, 3 * NBW], f16)
                pz2 = phpool.tile([D, 2 * NBW], f32, tag="ph")
                for k in range(2):
                    nc.tensor.matmul(
                        pz2[:, k * NBW:(k + 1) * NBW],
                        wrexp[:, k * D:(k + 1) * D],
                        xT[:, bs],
                        start=True, stop=True,
                    )
                if br_zero:
                    nc.scalar.activation(s[:, 0:2 * NBW], pz2[:], Act.Sigmoid)
                else:
                    for k in range(2):
                        nc.scalar.activation(
                            s[:, k * NBW:(k + 1) * NBW],
                            pz2[:, k * NBW:(k + 1) * NBW],
                            Act.Sigmoid, bias=brexp[:, k:k + 1],
                        )
                pz = popool.tile([D, NBW], f32, tag="po")
                nc.tensor.matmul(
                    pz[:], wrexp[:, 2 * D:3 * D], xT[:, bs],
                    start=True, stop=True,
                )
                nc.scalar.activation(
                    s[:, 2 * NBW:3 * NBW], pz[:], Act.Sigmoid,
                    **({} if br_zero else {"bias": brexp[:, 2:3]}),
                )
                # s chunk layout (host WrExp col order):
                # chunk0 = [lvl0 | lvl3], chunk1 = [lvl1 | lvl4], chunk2 = [lvl2 | lvl5]
                # DVE operands must share a start partition, so the cross-half
                # combine goes through SBUF->SBUF DMA.
                c0 = s[:, 0:NBW]
                c1 = s[:, NBW:2 * NBW]
                c2 = s[:, 2 * NBW:3 * NBW]
                eng = nc.vector if use_dve else nc.gpsimd
                t1 = ppool.tile([D, NBW], f16, tag="t1")
                eng.tensor_tensor(t1[:], c0, c1, Alu.mult)     # [l01 | l34]
                t2 = ppool.tile([D, NBW], f16, tag="t2")
                eng.tensor_tensor(t2[:], t1[:], c2, Alu.mult)  # [l012 | l345]
                u = ppool.tile([H, NBW], f16, tag="u")
                nc.sync.dma_start(u[:], t2[H:2 * H, :])
                w = ppool.tile([H, NBW], f16, tag="w")
                eng.tensor_tensor(w[:], t2[0:H, :], u[:], Alu.mult)  # path^T
                path2 = ppool.tile([D, NBW], f16, tag="path2")
                nc.sync.dma_start(path2[0:H, :], w[:])
                nc.sync.dma_start(path2[H:2 * H, :], w[:])
                return w, path2

            def emit_casts(pending):
                # 4 partition-aligned PSUM->SBUF casts (fp16), split between
                # ACT and DVE, then 4 strided DMAs out
                ppo, pnb = pending
                pbs = slice(pnb * NBW, (pnb + 1) * NBW)
                ot = otpool.tile([D, NBW], f16)
                for g in range(4):
                    if g < 2:
                        nc.scalar.activation(
                            ot[32 * g:32 * g + O, :],
                            ppo[32 * g:32 * g + O, :], Act.Copy)
                    else:
                        nc.vector.tensor_copy(
                            ot[32 * g:32 * g + O, :],
                            ppo[32 * g:32 * g + O, :])
                for g in range(4):
                    nc.sync.dma_start(
                        outT_d[O * g:O * (g + 1), pbs],
                        ot[32 * g:32 * g + O, :],
                    )

            # HAM warmup: keep the PE busy on scratch data while the input
            # DMAs land, so real matmuls start at 2.4 GHz
            scratch = cpool.tile([D, NBW], f16)
            nc.vector.memset(scratch[:], 0.0)
            pwarm = popool.tile([D, NBW], f32, tag="po")
            for _ in range(8):
                nc.tensor.matmul(pwarm[0:D, :], scratch[:, 0:D], scratch[:],
                                 start=True, stop=True)

            pending = None
            cur = emit_router(0, use_dve=True)
            for nb in range(NB):
                bs = slice(nb * NBW, (nb + 1) * NBW)
                w, path2 = cur

                # G2 is interleaved into the cg loop, delayed by G2_LAG units,
                # so PE never queues behind the full evacuation.
                G2_LAG = 2
                po = popool.tile([D, NBW], f32, tag="po")

                def emit_g2_unit(cgu):
                    for c in (2 * cgu, 2 * cgu + 1):
                        g = c % 4
                        hwt, hoff = hw_of[c]
                        nc.tensor.matmul(
                            po[32 * g:32 * g + O, :],
                            w2p[:, O * c:O * (c + 1)],
                            hwt[:, hoff:hoff + NBW],
                            start=(c < 4),
                            stop=(c >= NCH - 4 and (b2_zero or g != 0)),
                            tile_position=(0, 32 * g),
                        )

                # G1 (+ fused evacuation) over 16 cg units = 32 k' chunks.
                # ACT-route units are paired so the DVE path-multiply runs one
                # FD=2048 tensor_tensor per two units. G2 units are emitted
                # from a queue once their hw producer is emitted.
                hw_of = {}            # chunk c -> (tile, col offset)
                mult_done = set()     # cg units whose hw writes are emitted
                act_accum = None      # (hr2, hw2, [cg...])
                next_g2 = 0

                def flush_pair():
                    nonlocal act_accum
                    if act_accum is None:
                        return
                    hr2, hw2, cgs = act_accum
                    a = 2 * len(cgs)
                    p2b = path2[:].rearrange(
                        "p (one f) -> p one f", one=1).broadcast_to([D, a, NBW])
                    hr3 = hr2[:, 0:a * NBW].rearrange("p (a f) -> p a f", a=a)
                    hw3 = hw2[:, 0:a * NBW].rearrange("p (a f) -> p a f", a=a)
                    nc.vector.tensor_tensor(hw3, hr3, p2b, Alu.mult)
                    mult_done.update(cgs)
                    act_accum = None

                for cg in range(16):
                    ph = phpool.tile([D, 2 * NBW], f32, tag="ph")
                    for half in range(2):
                        c = 2 * cg + half
                        nc.tensor.matmul(
                            ph[:, half * NBW:(half + 1) * NBW],
                            w1p[:, c * D:(c + 1) * D],
                            xT[:, bs],
                            start=True, stop=True,
                        )
                    if cg == 3 and nb + 1 < NB:
                        nxt = emit_router(nb + 1)
                    if cg == 1 and pending is not None:
                        emit_casts(pending)
                        pending = None
                    if b1_zero and cg in STT_SET:
                        hw = hwpool.tile([D, 2 * NBW], f16, tag="hw")
                        p2b = path2[:].rearrange(
                            "p (one f) -> p one f", one=1).broadcast_to([D, 2, NBW])
                        hw3 = hw[:].rearrange("p (a f) -> p a f", a=2)
                        ph3 = ph[:].rearrange("p (a f) -> p a f", a=2)
                        nc.vector.scalar_tensor_tensor(
                            hw3, ph3, 0.0, p2b, Alu.max, Alu.mult,
                        )
                        hw_of[2 * cg] = (hw, 0)
                        hw_of[2 * cg + 1] = (hw, NBW)
                        mult_done.add(cg)
                    else:
                        if act_accum is None:
                            hr2 = hrpool.tile([D, 4 * NBW], f16, tag="hr")
                            hw2 = hwpool.tile([D, 4 * NBW], f16, tag="hw2")
                            act_accum = (hr2, hw2, [])
                        hr2, hw2, cgs = act_accum
                        off = len(cgs) * 2 * NBW
                        if b1_zero:
                            nc.scalar.activation(
                                hr2[:, off:off + 2 * NBW], ph[:], Act.Relu)
                        else:
                            for half in range(2):
                                c = 2 * cg + half
                                nc.scalar.activation(
                                    hr2[:, off + half * NBW:off + (half + 1) * NBW],
                                    ph[:, half * NBW:(half + 1) * NBW],
                                    Act.Relu, bias=b1p[:, c:c + 1],
                                )
                        hw_of[2 * cg] = (hw2, off)
                        hw_of[2 * cg + 1] = (hw2, off + NBW)
                        cgs.append(cg)
                        if len(cgs) == 2:
                            flush_pair()
                    while next_g2 <= cg - G2_LAG and next_g2 in mult_done:
                        emit_g2_unit(next_g2)
                        next_g2 += 1
                flush_pair()
                while next_g2 < 16:
                    emit_g2_unit(next_g2)
                    next_g2 += 1
                if not b2_zero:
                    nc.tensor.matmul(
                        po[0:O, :], b2s[:], w[:],
                        start=False, stop=True, tile_position=(0, 0),
                    )

                pending = (po, nb)
                if nb + 1 < NB:
                    cur = nxt
            emit_casts(pending)

    nc.compile()
    return nc


def _prep(x, Wr, br, W1, b1, W2, b2):
    """Host-side operand preparation (all fp16 except biases)."""
    chunk_levels = [(0, 3), (1, 4), (2, 5)]
    WrExp = np.zeros((D, 384), np.float32)
    brExp = np.zeros(384, np.float32)
    for k, levels in enumerate(chunk_levels):
        for half, lev in enumerate(levels):
            for l in range(L):
                q = 128 * k + 64 * half + l
                node = (2 ** lev - 1) + (l >> (DEPTH - lev))
                bit = (l >> (DEPTH - 1 - lev)) & 1
                sign = 1.0 - 2.0 * bit
                WrExp[:, q] = sign * Wr[node, :]
                brExp[q] = sign * br[node]

    W1p = W1.transpose(1, 2, 0).reshape(D, 4096)       # [d, j*64+l]
    W2p = W2.transpose(1, 0, 2).reshape(4096, O)       # [j*64+l, o]
    W2sb = W2p.reshape(NCH, D, O).transpose(1, 0, 2).reshape(D, NCH * O)
    b1p = b1.T.reshape(4096)                           # [j*64+l]

    shared = {
        "WrExp": np.ascontiguousarray(WrExp.astype(np.float16)),
        "W1p": np.ascontiguousarray(W1p.astype(np.float16)),
        "W2p": np.ascontiguousarray(W2sb.astype(np.float16)),
    }
    flags = (not np.any(brExp), not np.any(b1p), not np.any(b2))
    br_zero, b1_zero, b2_zero = flags
    if not b2_zero:
        shared["b2s"] = np.ascontiguousarray(b2.astype(np.float16))
    if not br_zero:
        shared["brExp"] = np.ascontiguousarray(brExp.reshape(3, D).T.astype(np.float32))
    if not b1_zero:
        shared["b1p"] = np.ascontiguousarray(b1p.reshape(NCH, D).T.astype(np.float32))
    return shared, flags


def kernel(x, Wr, br, W1, b1, W2, b2):
    from concourse.bass_utils import run_bass_kernel_spmd

    x = np.asarray(x, np.float32)
    Wr = np.asarray(Wr, np.float32)
    br = np.asarray(br, np.float32)
    W1 = np.asarray(W1, np.float32)
    b1 = np.asarray(b1, np.float32)
    W2 = np.asarray(W2, np.float32)
    b2 = np.asarray(b2, np.float32)

    shared, flags = _prep(x, Wr, br, W1, b1, W2, b2)
    if flags not in _cache:
        _cache[flags] = _build_module(*flags)
    nc = _cache[flags]

    in_maps = []
    for i in range(N_CORES):
        m = dict(shared)
        m["xT"] = np.ascontiguousarray(
            x[i * BC:(i + 1) * BC].T.astype(np.float16)
        )
        in_maps.append(m)

    res = run_bass_kernel_spmd(nc, in_maps, list(range(N_CORES)))
    out = np.empty((B, O), np.float32)
    for i in range(N_CORES):
        p4 = res.results[i]["outT4"].astype(np.float32)
        out[i * BC:(i + 1) * BC] = p4.reshape(4, O, BC).sum(axis=0).T
    return out


# revision 31
# speedup vs baseline: 1.3384x; 1.0813x over previous
"""Trainium2 Bass kernel for the soft neural decision tree (moe_routing).

Math (per batch row b):
  z~[q]   = sign(q) * (Wr[node(q)] . x[b])            q = (level, leaf) expanded
  path[l] = prod_level sigma(z~)                      (1 - sigma(z) = sigma(-z))
  h       = relu(x @ W1cat)                           W1cat in (j, l) interleaved order
  hw      = h * path[l(k')]                           broadcast over j
  out     = hw @ W2cat + path @ b2

Sharding: data-parallel over batch, 8 cores x 4096 rows. All GEMM inputs are
fp16 (1 cycle/row on the PE; fp32 is 4). The router/path stage for chunk
nb+1 is emitted mid-chunk so its latency chain hides under GEMM work.
PSUM evacuation of h is split between ScalarE (fused relu+cast) and VectorE
(fused scalar_tensor_tensor (h max 0) * path) to balance engines; the path
multiply runs on VectorE as fp16 tensor_tensor at 2x with a 0-stride
broadcast AP for the path operand. GEMM2 is k-split over 4 PE column-tile
groups whose partial sums are combined on the host.
"""

import numpy as np

N_CORES = 8
B = 32768
BC = B // N_CORES          # batch rows per core
NB = 8                     # batch sub-chunks per core
NBW = BC // NB             # 1024 columns per sub-chunk
D = 128
H = 64
L = 64
O = 10
DEPTH = 6
NCH = 32                   # k' chunks of 128 rows
N_STT = 5                  # cg units (of 16 per nb) evacuated via DVE STT route
# spread the DVE-routed units through the loop so ACT and DVE overlap
STT_SET = {(i * 16) // N_STT + 1 for i in range(N_STT)}

_cache = {}


def _build_module(br_zero, b1_zero, b2_zero):
    from concourse import bacc, tile
    import concourse.mybir as mybir

    f16 = mybir.dt.float16
    f32 = mybir.dt.float32
    Alu = mybir.AluOpType
    Act = mybir.ActivationFunctionType

    nc = bacc.Bacc(None)
    xT_d = nc.declare_dram_parameter("xT", [D, BC], f16, isOutput=False)
    wrexp_d = nc.declare_dram_parameter("WrExp", [D, 384], f16, isOutput=False)
    w1p_d = nc.declare_dram_parameter("W1p", [D, 4096], f16, isOutput=False)
    w2p_d = nc.declare_dram_parameter("W2p", [D, NCH * O], f16, isOutput=False)
    if not b2_zero:
        b2s_d = nc.declare_dram_parameter("b2s", [H, O], f16, isOutput=False)
    if not br_zero:
        brexp_d = nc.declare_dram_parameter("brExp", [D, 3], f32, isOutput=False)
    if not b1_zero:
        b1p_d = nc.declare_dram_parameter("b1p", [D, NCH], f32, isOutput=False)
    # 4 col-tiling partial sums per output element; summed on the host
    outT_d = nc.declare_dram_parameter("outT4", [4 * O, BC], f16, isOutput=True)

    with tile.TileContext(nc) as tc:
        with (
            tc.tile_pool(name="const", bufs=1) as cpool,
            tc.tile_pool(name="s", bufs=2) as spool,
            tc.tile_pool(name="path", bufs=2) as ppool,
            tc.tile_pool(name="hr", bufs=4) as hrpool,
            tc.tile_pool(name="hw", bufs=8) as hwpool,
            tc.tile_pool(name="ot", bufs=2) as otpool,
            tc.tile_pool(name="ph", bufs=3, space="PSUM") as phpool,
            tc.tile_pool(name="po", bufs=2, space="PSUM") as popool,
        ):
            xT = cpool.tile([D, BC], f16)
            wrexp = cpool.tile([D, 384], f16)
            w1p = cpool.tile([D, 4096], f16)
            w2p = cpool.tile([D, NCH * O], f16)
            nc.sync.dma_start(xT[:, 0:NBW], xT_d[:, 0:NBW])
            nc.gpsimd.dma_start(w1p[:, 0:1024], w1p_d[:, 0:1024])
            nc.sync.dma_start(wrexp[:], wrexp_d[:])
            for q in range(1, 4):
                nc.scalar.dma_start(w1p[:, q * 1024:(q + 1) * 1024],
                                    w1p_d[:, q * 1024:(q + 1) * 1024])
            nc.sync.dma_start(w2p[:], w2p_d[:])
            for q in range(1, NB):
                (nc.sync if q % 2 else nc.gpsimd).dma_start(
                    xT[:, q * NBW:(q + 1) * NBW],
                    xT_d[:, q * NBW:(q + 1) * NBW])
            if not b2_zero:
                b2s = cpool.tile([H, O], f16)
                nc.sync.dma_start(b2s[:], b2s_d[:])
            if not br_zero:
                brexp = cpool.tile([D, 3], f32)
                nc.sync.dma_start(brexp[:], brexp_d[:])
            if not b1_zero:
                b1p = cpool.tile([D, NCH], f32)
                nc.sync.dma_start(b1p[:], b1p_d[:])

            def emit_router(nb, use_dve=False):
                """G0 + sigmoid + path products for batch chunk nb.
                Returns (w, path2): path^T [64, NBW] and its 2x partition
                replica [128, NBW], both fp16."""
                bs = slice(nb * NBW, (nb + 1) * NBW)
                s = spool.tile([D, 3 * NBW], f16)
                pz2 = phpool.tile([D, 2 * NBW], f32, tag="ph")
                for k in range(2):
                    nc.tensor.matmul(
                        pz2[:, k * NBW:(k + 1) * NBW],
                        wrexp[:, k * D:(k + 1) * D],
                        xT[:, bs],
                        start=True, stop=True,
                    )
                if br_zero:
                    nc.scalar.activation(s[:, 0:2 * NBW], pz2[:], Act.Sigmoid)
                else:
                    for k in range(2):
                        nc.scalar.activation(
                            s[:, k * NBW:(k + 1) * NBW],
                            pz2[:, k * NBW:(k + 1) * NBW],
                            Act.Sigmoid, bias=brexp[:, k:k + 1],
                        )
                pz = popool.tile([D, NBW], f32, tag="po")
                nc.tensor.matmul(
                    pz[:], wrexp[:, 2 * D:3 * D], xT[:, bs],
                    start=True, stop=True,
                )
                nc.scalar.activation(
                    s[:, 2 * NBW:3 * NBW], pz[:], Act.Sigmoid,
                    **({} if br_zero else {"bias": brexp[:, 2:3]}),
                )
                # s chunk layout (host WrExp col order):
                # chunk0 = [lvl0 | lvl3], chunk1 = [lvl1 | lvl4], chunk2 = [lvl2 | lvl5]
                # DVE operands must share a start partition, so the cross-half
                # combine goes through SBUF->SBUF DMA.
                c0 = s[:, 0:NBW]
                c1 = s[:, NBW:2 * NBW]
                c2 = s[:, 2 * NBW:3 * NBW]
                eng = nc.vector if use_dve else nc.gpsimd
                t1 = ppool.tile([D, NBW], f16, tag="t1")
                eng.tensor_tensor(t1[:], c0, c1, Alu.mult)     # [l01 | l34]
                t2 = ppool.tile([D, NBW], f16, tag="t2")
                eng.tensor_tensor(t2[:], t1[:], c2, Alu.mult)  # [l012 | l345]
                u = ppool.tile([H, NBW], f16, tag="u")
                nc.sync.dma_start(u[:], t2[H:2 * H, :])
                w = ppool.tile([H, NBW], f16, tag="w")
                eng.tensor_tensor(w[:], t2[0:H, :], u[:], Alu.mult)  # path^T
                path2 = ppool.tile([D, NBW], f16, tag="path2")
                nc.sync.dma_start(path2[0:H, :], w[:])
                nc.sync.dma_start(path2[H:2 * H, :], w[:])
                return w, path2

            def emit_casts(pending):
                # 4 partition-aligned PSUM->SBUF casts (fp16), split between
                # ACT and DVE, then 4 strided DMAs out
                ppo, pnb = pending
                pbs = slice(pnb * NBW, (pnb + 1) * NBW)
                ot = otpool.tile([D, NBW], f16)
                for g in range(4):
                    if g < 2:
                        nc.scalar.activation(
                            ot[32 * g:32 * g + O, :],
                            ppo[32 * g:32 * g + O, :], Act.Copy)
                    else:
                        nc.vector.tensor_copy(
                            ot[32 * g:32 * g + O, :],
                            ppo[32 * g:32 * g + O, :])
                for g in range(4):
                    nc.sync.dma_start(
                        outT_d[O * g:O * (g + 1), pbs],
                        ot[32 * g:32 * g + O, :],
                    )

            # HAM warmup: keep the PE busy on scratch data while the input
            # DMAs land, so real matmuls start at 2.4 GHz
            scratch = cpool.tile([D, NBW], f16)
            nc.vector.memset(scratch[:], 0.0)
            pwarm = popool.tile([D, NBW], f32, tag="po")
            for _ in range(8):
                nc.tensor.matmul(pwarm[0:D, :], scratch[:, 0:D], scratch[:],
                                 start=True, stop=True)

            pending = None
            cur = emit_router(0, use_dve=True)
            for nb in range(NB):
                bs = slice(nb * NBW, (nb + 1) * NBW)
                w, path2 = cur

                # G2 is interleaved into the cg loop, delayed by G2_LAG units,
                # so PE never queues behind the full evacuation.
                G2_LAG = 3
                po = popool.tile([D, NBW], f32, tag="po")

                def emit_g2_unit(cgu):
                    for c in (2 * cgu, 2 * cgu + 1):
                        g = c % 4
                        hwt, hoff = hw_of[c]
                        nc.tensor.matmul(
                            po[32 * g:32 * g + O, :],
                            w2p[:, O * c:O * (c + 1)],
                            hwt[:, hoff:hoff + NBW],
                            start=(c < 4),
                            stop=(c >= NCH - 4 and (b2_zero or g != 0)),
                            tile_position=(0, 32 * g),
                        )

                # G1 (+ fused evacuation) over 16 cg units = 32 k' chunks.
                # ACT-route units are paired so the DVE path-multiply runs one
                # FD=2048 tensor_tensor per two units. G2 units are emitted
                # from a queue once their hw producer is emitted.
                hw_of = {}            # chunk c -> (tile, col offset)
                mult_done = set()     # cg units whose hw writes are emitted
                act_accum = None      # (hr2, hw2, [cg...])
                next_g2 = 0

                def flush_pair():
                    nonlocal act_accum
                    if act_accum is None:
                        return
                    hr2, hw2, cgs = act_accum
                    a = 2 * len(cgs)
                    p2b = path2[:].rearrange(
                        "p (one f) -> p one f", one=1).broadcast_to([D, a, NBW])
                    hr3 = hr2[:, 0:a * NBW].rearrange("p (a f) -> p a f", a=a)
                    hw3 = hw2[:, 0:a * NBW].rearrange("p (a f) -> p a f", a=a)
                    nc.vector.tensor_tensor(hw3, hr3, p2b, Alu.mult)
                    mult_done.update(cgs)
                    act_accum = None

                for cg in range(16):
                    ph = phpool.tile([D, 2 * NBW], f32, tag="ph")
                    for half in range(2):
                        c = 2 * cg + half
                        nc.tensor.matmul(
                            ph[:, half * NBW:(half + 1) * NBW],
                            w1p[:, c * D:(c + 1) * D],
                            xT[:, bs],
                            start=True, stop=True,
                        )
                    if cg == 3 and nb + 1 < NB:
                        nxt = emit_router(nb + 1)
                    if cg == 1 and pending is not None:
                        emit_casts(pending)
                        pending = None
                    if b1_zero and cg in STT_SET:
                        hw = hwpool.tile([D, 2 * NBW], f16, tag="hw")
                        p2b = path2[:].rearrange(
                            "p (one f) -> p one f", one=1).broadcast_to([D, 2, NBW])
                        hw3 = hw[:].rearrange("p (a f) -> p a f", a=2)
                        ph3 = ph[:].rearrange("p (a f) -> p a f", a=2)
                        nc.vector.scalar_tensor_tensor(
                            hw3, ph3, 0.0, p2b, Alu.max, Alu.mult,
                        )
                        hw_of[2 * cg] = (hw, 0)
                        hw_of[2 * cg + 1] = (hw, NBW)
                        mult_done.add(cg)
                    else:
                        if act_accum is None:
                            hr2 = hrpool.tile([D, 4 * NBW], f16, tag="hr")
                            hw2 = hwpool.tile([D, 4 * NBW], f16, tag="hw2")
                            act_accum = (hr2, hw2, [])
                        hr2, hw2, cgs = act_accum
                        off = len(cgs) * 2 * NBW
                        if b1_zero:
                            nc.scalar.activation(
                                hr2[:, off:off + 2 * NBW], ph[:], Act.Relu)
                        else:
                            for half in range(2):
                                c = 2 * cg + half
                                nc.scalar.activation(
                                    hr2[:, off + half * NBW:off + (half + 1) * NBW],
                                    ph[:, half * NBW:(half + 1) * NBW],
                                    Act.Relu, bias=b1p[:, c:c + 1],
                                )
                        hw_of[2 * cg] = (hw2, off)
                        hw_of[2 * cg + 1] = (hw2, off + NBW)
                        cgs.append(cg)
                        if len(cgs) == 2:
                            flush_pair()
                    while next_g2 <= cg - G2_LAG and next_g2 in mult_done:
                        emit_g2_unit(next_g2)
                        next_g2 += 1
                flush_pair()
                while next_g2 < 16:
                    emit_g2_unit(next_g2)
                    next_g2 += 1
                if not b2_zero:
                    nc.tensor.matmul(
                        po[0:O, :], b2s[:], w[:],
                        start=False, stop=True, tile_position=(0, 0),
                    )

                pending = (po, nb)
                if nb + 1 < NB:
                    cur = nxt
            emit_casts(pending)

    nc.compile()
    return nc


def _prep(x, Wr, br, W1, b1, W2, b2):
    """Host-side operand preparation (all fp16 except biases)."""
    chunk_levels = [(0, 3), (1, 4), (2, 5)]
    WrExp = np.zeros((D, 384), np.float32)
    brExp = np.zeros(384, np.float32)
    for k, levels in enumerate(chunk_levels):
        for half, lev in enumerate(levels):
            for l in range(L):
                q = 128 * k + 64 * half + l
                node = (2 ** lev - 1) + (l >> (DEPTH - lev))
                bit = (l >> (DEPTH - 1 - lev)) & 1
                sign = 1.0 - 2.0 * bit
                WrExp[:, q] = sign * Wr[node, :]
                brExp[q] = sign * br[node]

    W1p = W1.transpose(1, 2, 0).reshape(D, 4096)       # [d, j*64+l]
    W2p = W2.transpose(1, 0, 2).reshape(4096, O)       # [j*64+l, o]
    W2sb = W2p.reshape(NCH, D, O).transpose(1, 0, 2).reshape(D, NCH * O)
    b1p = b1.T.reshape(4096)                           # [j*64+l]

    shared = {
        "WrExp": np.ascontiguousarray(WrExp.astype(np.float16)),
        "W1p": np.ascontiguousarray(W1p.astype(np.float16)),
        "W2p": np.ascontiguousarray(W2sb.astype(np.float16)),
    }
    flags = (not np.any(brExp), not np.any(b1p), not np.any(b2))
    br_zero, b1_zero, b2_zero = flags
    if not b2_zero:
        shared["b2s"] = np.ascontiguousarray(b2.astype(np.float16))
    if not br_zero:
        shared["brExp"] = np.ascontiguousarray(brExp.reshape(3, D).T.astype(np.float32))
    if not b1_zero:
        shared["b1p"] = np.ascontiguousarray(b1p.reshape(NCH, D).T.astype(np.float32))
    return shared, flags


def kernel(x, Wr, br, W1, b1, W2, b2):
    from concourse.bass_utils import run_bass_kernel_spmd

    x = np.asarray(x, np.float32)
    Wr = np.asarray(Wr, np.float32)
    br = np.asarray(br, np.float32)
    W1 = np.asarray(W1, np.float32)
    b1 = np.asarray(b1, np.float32)
    W2 = np.asarray(W2, np.float32)
    b2 = np.asarray(b2, np.float32)

    shared, flags = _prep(x, Wr, br, W1, b1, W2, b2)
    if flags not in _cache:
        _cache[flags] = _build_module(*flags)
    nc = _cache[flags]

    in_maps = []
    for i in range(N_CORES):
        m = dict(shared)
        m["xT"] = np.ascontiguousarray(
            x[i * BC:(i + 1) * BC].T.astype(np.float16)
        )
        in_maps.append(m)

    res = run_bass_kernel_spmd(nc, in_maps, list(range(N_CORES)))
    out = np.empty((B, O), np.float32)
    for i in range(N_CORES):
        p4 = res.results[i]["outT4"].astype(np.float32)
        out[i * BC:(i + 1) * BC] = p4.reshape(4, O, BC).sum(axis=0).T
    return out
